# revision 1
# baseline (speedup 1.0000x reference)
"""Trainium2 Bass kernel for nn_Cross_SelfAttention (B=2, C=256, H=W=64, DQ=16).

Sharding: 8 cores = (batch b in {0,1}) x (attn stream s in {0,1}) x
(query half h in {0,1}).  Each core computes, for its (b, s):
    q = Wq @ x_s + bq   (only its query half i)
    k = Wk @ x_s        (bk dropped: constant-in-j terms cancel in softmax)
    S^T[j, i] = k[:, j] . q[:, i]
    E = exp(S^T) (no max subtraction; |S| <= ~15 so fp32/bf16 exp is safe)
    acc[st] = V_st^T-weighted sums of E columns (st = v1/v2 x 2 c-chunks)
    rowsum  = ones-stationary matmul over the same E
    o = (gamma*Wpt) @ ocat + bpt_eff  (bias via k=1 ones-row matmul)
    out = o * recip(rowsum) + x_residual
bv is folded into bpt_eff on the host (normalization makes the missing
V-bias contribution exactly Wpt @ [bv; bv]); gamma is folded into Wpt/bpt.

Each core writes a disjoint [256, 2048] slice of the output; no
collectives needed.
"""

import os

import numpy as np
import ml_dtypes

import concourse.bass as bass
import concourse.bacc as bacc
import concourse.mybir as mybir
from concourse.tile import TileContext
from concourse.bass import ts

BF16 = mybir.dt.bfloat16
F32 = mybir.dt.float32
F32R = mybir.dt.float32r

def _r(ap):
    """View an fp32 AP as float32r for full-rate PE matmuls (N>=256)."""
    return ap.bitcast(F32R)

B, C, HW, DQ = 2, 256, 4096, 16
HALF = HW // 2          # query positions per core
IB = 512                # i-block size (one PSUM bank at fp32)
N_IB = HALF // IB       # 4 i-blocks
N_JC = HW // 128        # 32 j-chunks

_NC_CACHE = {}

# Debug knob: repeat the main attention loop KREP times inside the program
# (device-time slope measurement through constant dispatch overhead).
KREP = int(os.environ.get("KREP", "1"))


def build_bass(krep=None):
    krep = KREP if krep is None else krep
    if krep in _NC_CACHE:
        return _NC_CACHE[krep]

    nc = bacc.Bacc("TRN2", target_bir_lowering=False, debug=False, num_devices=8)

    # Per-core inputs (full K/V range, query-half for q/residual).
    xq32_d = nc.dram_tensor("xq32", [C, HALF], F32, kind="ExternalInput")
    xk_d = nc.dram_tensor("xk32", [C, HW], F32R, kind="ExternalInput")
    # x streams pre-transposed on host: [HW, C], j on partitions after tiling
    xv1_d = nc.dram_tensor("xv1T", [HW, C], BF16, kind="ExternalInput")
    xv2_d = nc.dram_tensor("xv2T", [HW, C], BF16, kind="ExternalInput")
    # wq/wk replicated twice along M (cols 0:16 and 32:48) so S^T can use
    # 2x tile_position row-packing (contraction is only DQ=16 deep).
    wq_d = nc.dram_tensor("wqT", [C, 48], F32, kind="ExternalInput")
    wk_d = nc.dram_tensor("wkT", [C, 48], F32R, kind="ExternalInput")
    # wcat[r*C + c', c] = (gamma * Wpt[:, r-block] @ Wv)[c, c'] pre-composed
    # on host — the Wv projection and the output 1x1 conv fused into one.
    wcat_d = nc.dram_tensor("wcat", [2 * C, C], F32R, kind="ExternalInput")
    bq_d = nc.dram_tensor("bq_row", [1, 48], F32, kind="ExternalInput")
    bpt_d = nc.dram_tensor("bpt_col", [128, 2], F32, kind="ExternalInput")
    out_d = nc.dram_tensor("out", [C, HALF], F32, kind="ExternalOutput")

    with TileContext(nc) as tc:
        with (
            tc.tile_pool(name="persist", bufs=1) as pp,
            tc.tile_pool(name="work", bufs=1) as wp,
            tc.tile_pool(name="psum", bufs=1, space="PSUM") as psp,
        ):
            # ---- persistent SBUF tensors ----
            xq32 = pp.tile([128, 2, HALF], F32, name="xq32_sb")
            xk = pp.tile([128, 2, HW], F32R, name="xk_sb")
            xvt = [
                pp.tile([128, N_JC, C], BF16, name=f"xvt{r}_sb", tag=f"xvt{r}")
                for r in range(2)
            ]
            wq = pp.tile([128, 2, 48], F32, name="wq_sb")
            wk = pp.tile([128, 2, 48], F32R, name="wk_sb")
            wcat = pp.tile([128, 4, C], F32R, name="wcat_sb")
            bq = pp.tile([1, 48], F32, name="bq_sb")
            bpt = pp.tile([128, 2], F32, name="bpt_sb")
            ones_row = pp.tile([1, IB], F32, name="ones_row")
            ones128 = pp.tile([128, 128], BF16, name="ones128")
            qsb = pp.tile([48, HALF], F32R, name="qsb")
            ksb = pp.tile([48, HW], F32R, name="ksb")

            nc.vector.memset(ones_row[:], 1.0)
            nc.vector.memset(ones128[:], 1.0)

            # ---- load weights + x ----
            r128 = lambda ap: ap.rearrange("(o p) f -> p o f", p=128)
            nc.sync.dma_start(wq[:], r128(wq_d))
            nc.sync.dma_start(wk[:], r128(wk_d))
            nc.sync.dma_start(wcat[:], r128(wcat_d))
            nc.sync.dma_start(bq[:], bq_d[:])
            nc.sync.dma_start(bpt[:], bpt_d[:])
            for _rep in range(krep):
                nc.sync.dma_start(xq32[:], r128(xq32_d))
                nc.sync.dma_start(xk[:], r128(xk_d))
                nc.sync.dma_start(xvt[0][:], r128(xv1_d))
                nc.sync.dma_start(xvt[1][:], r128(xv2_d))

                # ---- Q projection (with bias), K projection (no bias) ----
                for p4 in range(N_IB):
                    q_ps = psp.tile([128, IB], F32, name="q_ps", tag="s", bufs=2)
                    nc.tensor.matmul(
                        q_ps[:48], wq[:, 0], xq32[:, 0, ts(p4, IB)],
                        start=True, stop=False,
                    )
                    nc.tensor.matmul(
                        q_ps[:48], wq[:, 1], xq32[:, 1, ts(p4, IB)],
                        start=False, stop=False,
                    )
                    nc.tensor.matmul(
                        q_ps[:48], bq[:], ones_row[:], start=False, stop=True,
                    )
                    nc.vector.tensor_copy(qsb[:, ts(p4, IB)], q_ps[:48])
                for p8 in range(HW // IB):
                    k_ps = psp.tile([128, IB], F32, name="k_ps", tag="s", bufs=2)
                    nc.tensor.matmul(
                        k_ps[:48], wk[:, 0], xk[:, 0, ts(p8, IB)],
                        start=True, stop=False,
                    )
                    nc.tensor.matmul(
                        k_ps[:48], wk[:, 1], xk[:, 1, ts(p8, IB)],
                        start=False, stop=True,
                    )
                    nc.vector.tensor_copy(ksb[:, ts(p8, IB)], k_ps[:48])

                # ---- main attention loop over i-blocks ----
                for ib in range(N_IB):
                    accs = [
                        psp.tile([128, IB], F32, name=f"acc{st}", tag="acc", bufs=5)
                        for st in range(4)
                    ]
                    acc1 = psp.tile([128, IB], F32, name="acc_ones", tag="acc", bufs=5)
                    for jp in range(N_JC // 2):
                        # two S^T chunks concurrently in PE row-groups 0 / 1
                        s_a = psp.tile([128, IB], F32, name="s_a", tag="s", bufs=2)
                        s_b = psp.tile([128, IB], F32, name="s_b", tag="s", bufs=2)
                        nc.tensor.matmul(
                            s_a[:], ksb[0:16, ts(2 * jp, 128)],
                            qsb[0:16, ts(ib, IB)],
                            start=True, stop=True, tile_position=(0, 0),
                        )
                        nc.tensor.matmul(
                            s_b[:], ksb[32:48, ts(2 * jp + 1, 128)],
                            qsb[32:48, ts(ib, IB)],
                            start=True, stop=True, tile_position=(32, 0),
                        )
                        for jc, s_ps in ((2 * jp, s_a), (2 * jp + 1, s_b)):
                            e_t = wp.tile([128, IB], BF16, name="e_t", tag="E", bufs=3)
                            nc.scalar.activation(
                                e_t[:], s_ps[:], mybir.ActivationFunctionType.Exp
                            )
                            for st in range(4):
                                nc.tensor.matmul(
                                    accs[st][:],
                                    xvt[st // 2][:, jc, ts(st % 2, 128)],
                                    e_t[:],
                                    start=(jc == 0), stop=(jc == N_JC - 1),
                                )
                            nc.tensor.matmul(
                                acc1[:], ones128[:], e_t[:],
                                start=(jc == 0), stop=(jc == N_JC - 1),
                            )

                    r_t = wp.tile([128, IB], F32, name="r_t", tag="R", bufs=2)
                    nc.vector.reciprocal(r_t[:], acc1[:])
                    ocat = wp.tile([128, 4, IB], F32R, name="ocat", tag="ocat", bufs=2)
                    for st in range(4):
                        nc.vector.tensor_copy(ocat[:, st], accs[st][:])

                    for cc in range(2):
                        p_ps = psp.tile([128, IB], F32, name="p_ps", tag="proj", bufs=1)
                        for cp in range(4):
                            nc.tensor.matmul(
                                p_ps[:], wcat[:, cp, ts(cc, 128)], ocat[:, cp],
                                start=(cp == 0), stop=(cp == 3),
                            )
                        o_t = wp.tile([128, IB], F32, name="o_t", tag="osb", bufs=3)
                        nc.vector.tensor_mul(o_t[:], p_ps[:], r_t[:])
                        # (o + bpt_eff) + x_residual; bpt is a per-partition scalar
                        nc.vector.scalar_tensor_tensor(
                            o_t[:], o_t[:], bpt[:, cc:cc + 1], xq32[:, cc, ts(ib, IB)],
                            op0=mybir.AluOpType.add, op1=mybir.AluOpType.add,
                        )
                        nc.sync.dma_start(
                            out_d.rearrange("(o p) f -> p o f", p=128)[:, cc, ts(ib, IB)],
                            o_t[:],
                        )

    nc.compile()
    _NC_CACHE[krep] = nc
    return nc


def _prep_maps(x, Wq, bq, Wk, bk, Wv, bv, Wpt, bpt, gamma):
    bf16 = ml_dtypes.bfloat16
    f32 = np.float32
    g = float(np.asarray(gamma).reshape(-1)[0])
    # wq/wk/bq replicated at column offsets 0 and 32 (S^T 2x row-packing)
    wqT = np.zeros((C, 48), f32)
    wqT[:, 0:DQ] = Wq.T
    wqT[:, 32:32 + DQ] = Wq.T
    wkT = np.zeros((C, 48), f32)
    wkT[:, 0:DQ] = Wk.T
    wkT[:, 32:32 + DQ] = Wk.T
    bq_row = np.zeros((1, 48), f32)
    bq_row[0, 0:DQ] = bq
    bq_row[0, 32:32 + DQ] = bq
    # fuse the Wv projection into the output 1x1 conv:
    #   o = sum_r (g*Wpt[:, r-block] @ Wv) @ (X_r E) ;  wcat rows = c' of X_r
    wpt_g = (g * Wpt).astype(f32)
    wcat = np.concatenate(
        [(wpt_g[:, :C] @ Wv).T, (wpt_g[:, C:] @ Wv).T], axis=0
    ).astype(f32)  # [2C, C]: row r*C+c', col c
    bpt_eff = (g * (bpt + Wpt @ np.concatenate([bv, bv]))).astype(np.float32)
    bpt_col = np.ascontiguousarray(bpt_eff.reshape(2, 128).T)

    xf = np.asarray(x, np.float32).reshape(B, 2, C, HW)
    in_maps = []
    for core in range(8):
        b, s, h = core >> 2, (core >> 1) & 1, core & 1
        in_maps.append(
            dict(
                xq32=np.ascontiguousarray(xf[b, s, :, h * HALF:(h + 1) * HALF]),
                xk32=np.ascontiguousarray(xf[b, s]),
                xv1T=np.ascontiguousarray(xf[b, 0].T.astype(bf16)),
                xv2T=np.ascontiguousarray(xf[b, 1].T.astype(bf16)),
                wqT=wqT, wkT=wkT, wcat=wcat,
                bq_row=bq_row, bpt_col=bpt_col,
            )
        )
    return in_maps


def kernel(x, Wq, bq, Wk, bk, Wv, bv, Wpt, bpt, gamma, _trace=False):
    from concourse.bass_utils import run_bass_kernel_spmd

    nc = build_bass()
    in_maps = _prep_maps(x, Wq, bq, Wk, bk, Wv, bv, Wpt, bpt, gamma)
    res = run_bass_kernel_spmd(nc, in_maps, list(range(8)), trace=_trace)

    out = np.empty((B, 2, C, HW), np.float32)
    for core in range(8):
        b, s, h = core >> 2, (core >> 1) & 1, core & 1
        out[b, s, :, h * HALF:(h + 1) * HALF] = res.results[core]["out"]
    full = out.reshape(B, 2 * C, 64, 64)
    if _trace:
        return full, res
    return full



# revision 3
# speedup vs baseline: 1.5391x; 1.5391x over previous
"""Trainium2 Bass kernel for nn_Cross_SelfAttention (B=2, C=256, H=W=64, DQ=16).

Sharding: 8 cores = (batch b) x (attn stream s) x (query half h).

Key algebraic restructure vs the v1 kernel: the output 1x1 conv is linear,
so  Wpt @ [attn@v1; attn@v2] = (Wpt1@V1 + Wpt2@V2) @ attn^T = M @ attn^T.
M = [C, HW] is shared by both attn streams of a batch and folds Wv, Wpt,
gamma and the 512->256 projection into ONE 256-channel attention apply:
half the PE work of applying attention to v1 and v2 separately, and no
output-projection matmuls at all.

Per core:
    M^T[j, c] = x1^T @ wcat1 + x2^T @ wcat2        (wcat_r = (g*Wpt_r@Wv)^T)
    k = Wk @ x_own (bf16; bk dropped, cancels in softmax)
    q = Wq @ x_own[:, half] + bq  (fp32 matmul from residual tile, ACT bias)
    S^T[j, i] = k[:, j] . q[:, i]   (pairs of j-chunks, 2-way row packing)
    E = exp(S^T)  (one scalar-engine ACT per 2-bank PSUM pair)
    acc[c, i] += M^T[jc, :128/128:]^T @ E           (2 c-chunks, PSUM)
    rs2 += E partial rowsums on DVE (bf16); ones-matmul reduces partitions
    out = acc * recip(rowsum) + bpt_eff + x_residual
bv is folded into bpt_eff on the host (normalization makes the missing
V-bias contribution exactly Wpt @ [bv; bv]); gamma is folded into wcat/bpt.
For s=1 cores the host swaps (x1b,x2b) AND (w1,w2) jointly — M is
invariant, and x1b is always the core's own attention stream.

Each core writes a disjoint [256, 2048] slice of the output; no
collectives needed.
"""

import os

import numpy as np
import ml_dtypes

import concourse.bass as bass
import concourse.bacc as bacc
import concourse.mybir as mybir
from concourse.tile import TileContext
from concourse.bass import ts

BF16 = mybir.dt.bfloat16
F32 = mybir.dt.float32

B, C, HW, DQ = 2, 256, 4096, 16
HALF = HW // 2          # query positions per core
IB = 512                # i-block size (one PSUM bank at fp32)
N_IB = HALF // IB       # 4 i-blocks
N_JC = HW // 128        # 32 j-chunks

_NC_CACHE = {}

# Debug knob: repeat the main attention loop KREP times inside the program
# (device-time slope measurement through constant dispatch overhead).
KREP = int(os.environ.get("KREP", "1"))


def build_bass(krep=None):
    krep = KREP if krep is None else krep
    if krep in _NC_CACHE:
        return _NC_CACHE[krep]

    nc = bacc.Bacc("TRN2", target_bir_lowering=False, debug=False, num_devices=8)

    # Per-core inputs.
    x1_d = nc.dram_tensor("x1b", [C, HW], BF16, kind="ExternalInput")
    x2_d = nc.dram_tensor("x2b", [C, HW], BF16, kind="ExternalInput")
    xr_d = nc.dram_tensor("xres", [C, HALF], F32, kind="ExternalInput")
    # wq/wk replicated twice along M (cols 0:16 and 32:48) so S^T can use
    # 2x tile_position row-packing (contraction is only DQ=16 deep).
    wq_d = nc.dram_tensor("wq2", [C, 48], F32, kind="ExternalInput")
    wk_d = nc.dram_tensor("wk2", [C, 48], BF16, kind="ExternalInput")
    # wcat[r*C + c', c] = (gamma * Wpt[:, r-block] @ Wv)[c, c'] pre-composed
    # on host — Wv, the output 1x1 conv and gamma fused into one matrix.
    w1_d = nc.dram_tensor("w1", [C, C], BF16, kind="ExternalInput")
    w2_d = nc.dram_tensor("w2", [C, C], BF16, kind="ExternalInput")
    bq_d = nc.dram_tensor("bq_col", [48, 1], F32, kind="ExternalInput")
    bpt_d = nc.dram_tensor("bpt_col", [128, 2], F32, kind="ExternalInput")
    out_d = nc.dram_tensor("out", [C, HALF], F32, kind="ExternalOutput")

    with TileContext(nc) as tc:
        with (
            tc.tile_pool(name="persist", bufs=1) as pp,
            tc.tile_pool(name="work", bufs=1) as wp,
            tc.tile_pool(name="psum", bufs=1, space="PSUM") as psp,
        ):
            # ---- persistent SBUF tensors ----
            x1 = pp.tile([128, 2, HW], BF16, name="x1_sb")
            x2 = pp.tile([128, 2, HW], BF16, name="x2_sb")
            xr = pp.tile([128, 2, HALF], F32, name="xr_sb")
            mT = pp.tile([128, N_JC, C], BF16, name="mT_sb")
            wq = pp.tile([128, 2, 48], F32, name="wq_sb")
            wk = pp.tile([128, 2, 48], BF16, name="wk_sb")
            w1s = pp.tile([128, 2, C], BF16, name="w1s_sb")
            w2s = pp.tile([128, 2, C], BF16, name="w2s_sb")
            bq = pp.tile([48, 1], F32, name="bq_sb")
            bpt = pp.tile([128, 2], F32, name="bpt_sb")
            ones128 = pp.tile([128, 128], BF16, name="ones128")
            qsb = pp.tile([48, HALF], BF16, name="qsb")
            ksb = pp.tile([48, HW], BF16, name="ksb")

            nc.vector.memset(ones128[:], 1.0)

            r128 = lambda ap: ap.rearrange("(o p) f -> p o f", p=128)
            nc.sync.dma_start(wq[:], r128(wq_d))
            nc.sync.dma_start(wk[:], r128(wk_d))
            nc.sync.dma_start(w1s[:], r128(w1_d))
            nc.sync.dma_start(w2s[:], r128(w2_d))
            nc.sync.dma_start(bq[:], bq_d[:])
            nc.sync.dma_start(bpt[:], bpt_d[:])
            for _rep in range(krep):
                # x1/x2 in 4 column-chunks each, interleaved, so M^T compute
                # can start before the full 4MB lands.
                for q4 in range(4):
                    nc.sync.dma_start(
                        x1[:, :, ts(q4, 1024)], r128(x1_d)[:, :, ts(q4, 1024)]
                    )
                    nc.sync.dma_start(
                        x2[:, :, ts(q4, 1024)], r128(x2_d)[:, :, ts(q4, 1024)]
                    )
                nc.sync.dma_start(xr[:], r128(xr_d))

                # ---- M^T = x1^T @ wcat1 + x2^T @ wcat2, [j, c] bf16 ----
                for jc in range(N_JC):
                    m_ps = psp.tile([128, IB], F32, name="m_ps", tag="acc", bufs=4)
                    for cp in range(4):
                        xs_ = x1 if cp < 2 else x2
                        ws_ = w1s if cp < 2 else w2s
                        o = cp % 2
                        nc.tensor.matmul(
                            m_ps[:, 0:C], xs_[:, o, ts(jc, 128)], ws_[:, o],
                            start=(cp == 0), stop=(cp == 3),
                        )
                    nc.vector.tensor_copy(mT[:, jc, :], m_ps[:, 0:C])

                # ---- K projection (no bias), bf16 out; x1 is the own stream
                for p4 in range(4):
                    k_ps = psp.tile([128, 2, IB], F32, name="k_ps", tag="s", bufs=2)
                    for hf in range(2):
                        p8 = 2 * p4 + hf
                        nc.tensor.matmul(
                            k_ps[:48, hf], wk[:, 0], x1[:, 0, ts(p8, IB)],
                            start=True, stop=False,
                        )
                        nc.tensor.matmul(
                            k_ps[:48, hf], wk[:, 1], x1[:, 1, ts(p8, IB)],
                            start=False, stop=True,
                        )
                        nc.scalar.copy(ksb[:, ts(p8, IB)], k_ps[:48, hf])

                # ---- Q projection from the fp32 residual tile (own i-half),
                # bias added during the PSUM->SBUF cast on the scalar engine.
                for p2 in range(2):
                    q_ps = psp.tile([128, 2, IB], F32, name="q_ps", tag="s", bufs=2)
                    for hf in range(2):
                        p4 = 2 * p2 + hf
                        nc.tensor.matmul(
                            q_ps[:48, hf], wq[:, 0], xr[:, 0, ts(p4, IB)],
                            start=True, stop=False,
                        )
                        nc.tensor.matmul(
                            q_ps[:48, hf], wq[:, 1], xr[:, 1, ts(p4, IB)],
                            start=False, stop=True,
                        )
                        nc.scalar.activation(
                            qsb[:, ts(p4, IB)], q_ps[:48, hf],
                            mybir.ActivationFunctionType.Identity, bias=bq[:],
                        )

                # ---- main attention loop over i-blocks ----
                for ib in range(N_IB):
                    acc0 = psp.tile([128, IB], F32, name="acc0", tag="acc", bufs=4)
                    acc1c = psp.tile([128, IB], F32, name="acc1c", tag="acc", bufs=4)
                    rs2 = wp.tile([128, 2, IB], BF16, name="rs2", tag="rs", bufs=2)
                    for p in range(N_JC // 2):
                        s_p = psp.tile([128, 2, IB], F32, name="s_p", tag="s", bufs=2)
                        nc.tensor.matmul(
                            s_p[:, 0], ksb[0:16, ts(2 * p, 128)],
                            qsb[0:16, ts(ib, IB)],
                            start=True, stop=True, tile_position=(0, 0),
                        )
                        nc.tensor.matmul(
                            s_p[:, 1], ksb[32:48, ts(2 * p + 1, 128)],
                            qsb[32:48, ts(ib, IB)],
                            start=True, stop=True, tile_position=(32, 0),
                        )
                        e_p = wp.tile([128, 2, IB], BF16, name="e_p", tag="E", bufs=3)
                        nc.scalar.activation(
                            e_p[:], s_p[:], mybir.ActivationFunctionType.Exp
                        )
                        for k2 in range(2):
                            jc = 2 * p + k2
                            nc.tensor.matmul(
                                acc0[:], mT[:, jc, 0:128], e_p[:, k2],
                                start=(jc == 0), stop=(jc == N_JC - 1),
                            )
                            nc.tensor.matmul(
                                acc1c[:], mT[:, jc, 128:256], e_p[:, k2],
                                start=(jc == 0), stop=(jc == N_JC - 1),
                            )
                        if p == 0:
                            nc.vector.tensor_copy(rs2[:], e_p[:])
                        else:
                            nc.vector.tensor_add(rs2[:], rs2[:], e_p[:])

                    # rowsum: reduce rs2 over partitions (broadcast to all 128)
                    acc_rs = psp.tile([128, IB], F32, name="acc_rs", tag="acc", bufs=4)
                    nc.tensor.matmul(acc_rs[:], ones128[:], rs2[:, 0], start=True, stop=False)
                    nc.tensor.matmul(acc_rs[:], ones128[:], rs2[:, 1], start=False, stop=True)
                    r_t = wp.tile([128, IB], F32, name="r_t", tag="R", bufs=2)
                    nc.vector.reciprocal(r_t[:], acc_rs[:])

                    for cc in range(2):
                        o_t = wp.tile([128, IB], F32, name="o_t", tag="osb", bufs=3)
                        acc_cc = acc0 if cc == 0 else acc1c
                        nc.vector.tensor_mul(o_t[:], acc_cc[:], r_t[:])
                        # (o + bpt_eff) + x_residual; bpt is per-partition
                        nc.vector.scalar_tensor_tensor(
                            o_t[:], o_t[:], bpt[:, cc:cc + 1], xr[:, cc, ts(ib, IB)],
                            op0=mybir.AluOpType.add, op1=mybir.AluOpType.add,
                        )
                        nc.sync.dma_start(
                            out_d.rearrange("(o p) f -> p o f", p=128)[:, cc, ts(ib, IB)],
                            o_t[:],
                        )

    nc.compile()
    _NC_CACHE[krep] = nc
    return nc


def _prep_maps(x, Wq, bq, Wk, bk, Wv, bv, Wpt, bpt, gamma):
    bf16 = ml_dtypes.bfloat16
    f32 = np.float32
    g = float(np.asarray(gamma).reshape(-1)[0])
    # wq/wk/bq replicated at column offsets 0 and 32 (S^T 2x row-packing)
    wq2 = np.zeros((C, 48), f32)
    wq2[:, 0:DQ] = Wq.T
    wq2[:, 32:32 + DQ] = Wq.T
    wk2 = np.zeros((C, 48), f32)
    wk2[:, 0:DQ] = Wk.T
    wk2[:, 32:32 + DQ] = Wk.T
    bq_col = np.zeros((48, 1), f32)
    bq_col[0:DQ, 0] = bq
    bq_col[32:32 + DQ, 0] = bq
    # wcat_r = (g * Wpt[:, r-block] @ Wv).T, layout [c', c]
    wpt_g = (g * Wpt).astype(f32)
    wcat1 = (wpt_g[:, :C] @ Wv).T.astype(f32)
    wcat2 = (wpt_g[:, C:] @ Wv).T.astype(f32)
    bpt_eff = (g * (bpt + Wpt @ np.concatenate([bv, bv]))).astype(f32)
    bpt_col = np.ascontiguousarray(bpt_eff.reshape(2, 128).T)

    xf = np.asarray(x, f32).reshape(B, 2, C, HW)
    xb = xf.astype(bf16)
    wk2b = wk2.astype(bf16)
    w1b, w2b = wcat1.astype(bf16), wcat2.astype(bf16)
    in_maps = []
    for core in range(8):
        b, s, h = core >> 2, (core >> 1) & 1, core & 1
        # joint (x1,x2)/(w1,w2) swap for s=1: M invariant, x1b = own stream
        in_maps.append(
            dict(
                x1b=np.ascontiguousarray(xb[b, s]),
                x2b=np.ascontiguousarray(xb[b, 1 - s]),
                xres=np.ascontiguousarray(xf[b, s, :, h * HALF:(h + 1) * HALF]),
                wq2=wq2, wk2=wk2b,
                w1=(w1b if s == 0 else w2b),
                w2=(w2b if s == 0 else w1b),
                bq_col=bq_col, bpt_col=bpt_col,
            )
        )
    return in_maps


def kernel(x, Wq, bq, Wk, bk, Wv, bv, Wpt, bpt, gamma, _trace=False):
    from concourse.bass_utils import run_bass_kernel_spmd

    nc = build_bass()
    in_maps = _prep_maps(x, Wq, bq, Wk, bk, Wv, bv, Wpt, bpt, gamma)
    res = run_bass_kernel_spmd(nc, in_maps, list(range(8)), trace=_trace)

    out = np.empty((B, 2, C, HW), np.float32)
    for core in range(8):
        b, s, h = core >> 2, (core >> 1) & 1, core & 1
        out[b, s, :, h * HALF:(h + 1) * HALF] = res.results[core]["out"]
    full = out.reshape(B, 2 * C, 64, 64)
    if _trace:
        return full, res
    return full


# revision 4
# speedup vs baseline: 1.6069x; 1.0440x over previous
"""Trainium2 Bass kernel for nn_Cross_SelfAttention (B=2, C=256, H=W=64, DQ=16).

Sharding: 8 cores = (batch b) x (attn stream s) x (query half h).

Key algebraic restructure vs the v1 kernel: the output 1x1 conv is linear,
so  Wpt @ [attn@v1; attn@v2] = (Wpt1@V1 + Wpt2@V2) @ attn^T = M @ attn^T.
M = [C, HW] is shared by both attn streams of a batch and folds Wv, Wpt,
gamma and the 512->256 projection into ONE 256-channel attention apply:
half the PE work of applying attention to v1 and v2 separately, and no
output-projection matmuls at all.

Per core:
    M^T[j, c] = x1^T @ wcat1 + x2^T @ wcat2        (wcat_r = (g*Wpt_r@Wv)^T)
    k = Wk @ x_own (bf16; bk dropped, cancels in softmax)
    q = Wq @ x_own[:, half] + bq  (fp32 matmul from residual tile, ACT bias)
    S^T[j, i] = k[:, j] . q[:, i]   (pairs of j-chunks, 2-way row packing)
    E = exp(S^T)  (one scalar-engine ACT per 2-bank PSUM pair)
    acc[c, i] += M^T[jc, :128/128:]^T @ E           (2 c-chunks, PSUM)
    rs2 += E partial rowsums on DVE (bf16); ones-matmul reduces partitions
    out = acc * recip(rowsum) + bpt_eff + x_residual
bv is folded into bpt_eff on the host (normalization makes the missing
V-bias contribution exactly Wpt @ [bv; bv]); gamma is folded into wcat/bpt.
For s=1 cores the host swaps (x1b,x2b) AND (w1,w2) jointly — M is
invariant, and x1b is always the core's own attention stream.

Each core writes a disjoint [256, 2048] slice of the output; no
collectives needed.
"""

import os

import numpy as np
import ml_dtypes

import concourse.bass as bass
import concourse.bacc as bacc
import concourse.mybir as mybir
from concourse.tile import TileContext
from concourse.bass import ts

BF16 = mybir.dt.bfloat16
F32 = mybir.dt.float32

B, C, HW, DQ = 2, 256, 4096, 16
HALF = HW // 2          # query positions per core
IB = 512                # i-block size (one PSUM bank at fp32)
N_IB = HALF // IB       # 4 i-blocks
N_JC = HW // 128        # 32 j-chunks

_NC_CACHE = {}

# Debug knob: repeat the main attention loop KREP times inside the program
# (device-time slope measurement through constant dispatch overhead).
KREP = int(os.environ.get("KREP", "1"))


def build_bass(krep=None):
    krep = KREP if krep is None else krep
    if krep in _NC_CACHE:
        return _NC_CACHE[krep]

    nc = bacc.Bacc("TRN2", target_bir_lowering=False, debug=False, num_devices=8)

    # Per-core inputs.
    x1_d = nc.dram_tensor("x1b", [C, HW], BF16, kind="ExternalInput")
    x2_d = nc.dram_tensor("x2b", [C, HW], BF16, kind="ExternalInput")
    xr_d = nc.dram_tensor("xres", [C, HALF], F32, kind="ExternalInput")
    # wq/wk replicated twice along M (cols 0:16 and 32:48) so S^T can use
    # 2x tile_position row-packing (contraction is only DQ=16 deep).
    wq_d = nc.dram_tensor("wq2", [C, 48], F32, kind="ExternalInput")
    wk_d = nc.dram_tensor("wk2", [C, 48], BF16, kind="ExternalInput")
    # wcat[r*C + c', c] = (gamma * Wpt[:, r-block] @ Wv)[c, c'] pre-composed
    # on host — Wv, the output 1x1 conv and gamma fused into one matrix.
    w1_d = nc.dram_tensor("w1", [C, C], BF16, kind="ExternalInput")
    w2_d = nc.dram_tensor("w2", [C, C], BF16, kind="ExternalInput")
    bq_d = nc.dram_tensor("bq_col", [48, 1], F32, kind="ExternalInput")
    bpt_d = nc.dram_tensor("bpt_col", [128, 2], F32, kind="ExternalInput")
    out_d = nc.dram_tensor("out", [C, HALF], F32, kind="ExternalOutput")

    with TileContext(nc) as tc:
        with (
            tc.tile_pool(name="persist", bufs=1) as pp,
            tc.tile_pool(name="work", bufs=1) as wp,
            tc.tile_pool(name="psum", bufs=1, space="PSUM") as psp,
        ):
            # ---- persistent SBUF tensors ----
            x1 = pp.tile([128, 2, HW], BF16, name="x1_sb")
            x2 = pp.tile([128, 2, HW], BF16, name="x2_sb")
            xr = pp.tile([128, 2, HALF], F32, name="xr_sb")
            mT = pp.tile([128, N_JC, C], BF16, name="mT_sb")
            wq = pp.tile([128, 2, 48], F32, name="wq_sb")
            wk = pp.tile([128, 2, 48], BF16, name="wk_sb")
            w1s = pp.tile([128, 2, C], BF16, name="w1s_sb")
            w2s = pp.tile([128, 2, C], BF16, name="w2s_sb")
            bq = pp.tile([48, 1], F32, name="bq_sb")
            bpt = pp.tile([128, 2], F32, name="bpt_sb")
            ones128 = pp.tile([128, 128], BF16, name="ones128")
            qsb = pp.tile([48, HALF], BF16, name="qsb")
            ksb = pp.tile([48, HW], BF16, name="ksb")

            nc.vector.memset(ones128[:], 1.0)

            r128 = lambda ap: ap.rearrange("(o p) f -> p o f", p=128)
            nc.sync.dma_start(wq[:], r128(wq_d))
            nc.sync.dma_start(wk[:], r128(wk_d))
            nc.sync.dma_start(w1s[:], r128(w1_d))
            nc.sync.dma_start(w2s[:], r128(w2_d))
            nc.sync.dma_start(bq[:], bq_d[:])
            nc.sync.dma_start(bpt[:], bpt_d[:])
            for _rep in range(krep):
                # x1/x2 in 4 column-chunks each, interleaved, so M^T compute
                # can start before the full 4MB lands.
                for q4 in range(4):
                    nc.sync.dma_start(
                        x1[:, :, ts(q4, 1024)], r128(x1_d)[:, :, ts(q4, 1024)]
                    )
                    nc.sync.dma_start(
                        x2[:, :, ts(q4, 1024)], r128(x2_d)[:, :, ts(q4, 1024)]
                    )
                nc.sync.dma_start(xr[:], r128(xr_d))

                # ---- M^T = x1^T @ wcat1 + x2^T @ wcat2, [j, c] bf16 ----
                for jc in range(N_JC):
                    m_ps = psp.tile([128, IB], F32, name="m_ps", tag="acc", bufs=4)
                    for cp in range(4):
                        xs_ = x1 if cp < 2 else x2
                        ws_ = w1s if cp < 2 else w2s
                        o = cp % 2
                        nc.tensor.matmul(
                            m_ps[:, 0:C], xs_[:, o, ts(jc, 128)], ws_[:, o],
                            start=(cp == 0), stop=(cp == 3),
                        )
                    nc.vector.tensor_copy(mT[:, jc, :], m_ps[:, 0:C])

                # ---- K projection (no bias), bf16 out; x1 is the own stream
                for p4 in range(4):
                    k_ps = psp.tile([128, 2, IB], F32, name="k_ps", tag="s", bufs=2)
                    for hf in range(2):
                        p8 = 2 * p4 + hf
                        nc.tensor.matmul(
                            k_ps[:48, hf], wk[:, 0], x1[:, 0, ts(p8, IB)],
                            start=True, stop=False,
                        )
                        nc.tensor.matmul(
                            k_ps[:48, hf], wk[:, 1], x1[:, 1, ts(p8, IB)],
                            start=False, stop=True,
                        )
                        nc.scalar.copy(ksb[:, ts(p8, IB)], k_ps[:48, hf])

                # ---- Q projection from the fp32 residual tile (own i-half),
                # bias added during the PSUM->SBUF cast on the scalar engine.
                for p2 in range(2):
                    q_ps = psp.tile([128, 2, IB], F32, name="q_ps", tag="s", bufs=2)
                    for hf in range(2):
                        p4 = 2 * p2 + hf
                        nc.tensor.matmul(
                            q_ps[:48, hf], wq[:, 0], xr[:, 0, ts(p4, IB)],
                            start=True, stop=False,
                        )
                        nc.tensor.matmul(
                            q_ps[:48, hf], wq[:, 1], xr[:, 1, ts(p4, IB)],
                            start=False, stop=True,
                        )
                        nc.scalar.activation(
                            qsb[:, ts(p4, IB)], q_ps[:48, hf],
                            mybir.ActivationFunctionType.Identity, bias=bq[:],
                        )

                # ---- main attention loop: pairs of j-chunks, software-
                # pipelined one stage deep so S^T(p+1) overlaps exp(p).
                NP = N_JC // 2  # pairs per i-block

                def issue_st(g):
                    ib, p = divmod(g, NP)
                    s_p = psp.tile([128, 2, IB], F32, name="s_p", tag="s", bufs=2)
                    nc.tensor.matmul(
                        s_p[:, 0], ksb[0:16, ts(2 * p, 128)],
                        qsb[0:16, ts(ib, IB)],
                        start=True, stop=True, tile_position=(0, 0),
                    )
                    nc.tensor.matmul(
                        s_p[:, 1], ksb[32:48, ts(2 * p + 1, 128)],
                        qsb[32:48, ts(ib, IB)],
                        start=True, stop=True, tile_position=(32, 0),
                    )
                    return s_p

                accs = rs2 = None
                s_cur = issue_st(0)
                for g in range(N_IB * NP):
                    ib, p = divmod(g, NP)
                    if p == 0:
                        acc0 = psp.tile([128, IB], F32, name="acc0", tag="acc", bufs=4)
                        acc1c = psp.tile([128, IB], F32, name="acc1c", tag="acc", bufs=4)
                        rs2 = wp.tile([128, 2, IB], BF16, name="rs2", tag="rs", bufs=2)
                    e_p = wp.tile([128, 2, IB], BF16, name="e_p", tag="E", bufs=3)
                    nc.scalar.activation(
                        e_p[:], s_cur[:], mybir.ActivationFunctionType.Exp
                    )
                    if g + 1 < N_IB * NP:
                        s_cur = issue_st(g + 1)
                    for k2 in range(2):
                        jc = 2 * p + k2
                        nc.tensor.matmul(
                            acc0[:], mT[:, jc, 0:128], e_p[:, k2],
                            start=(jc == 0), stop=(jc == N_JC - 1),
                        )
                        nc.tensor.matmul(
                            acc1c[:], mT[:, jc, 128:256], e_p[:, k2],
                            start=(jc == 0), stop=(jc == N_JC - 1),
                        )
                    if p == 0:
                        nc.vector.tensor_copy(rs2[:], e_p[:])
                    else:
                        nc.vector.tensor_add(rs2[:], rs2[:], e_p[:])

                    if p == NP - 1:
                        # rowsum: reduce rs2 over partitions (bcast to all 128)
                        acc_rs = psp.tile([128, IB], F32, name="acc_rs", tag="acc", bufs=4)
                        nc.tensor.matmul(acc_rs[:], ones128[:], rs2[:, 0], start=True, stop=False)
                        nc.tensor.matmul(acc_rs[:], ones128[:], rs2[:, 1], start=False, stop=True)
                        r_t = wp.tile([128, IB], BF16, name="r_t", tag="R", bufs=2)
                        with nc.allow_low_precision(reason="recip in bf16: 0.4% on a multiplicative factor"):
                            nc.vector.reciprocal(r_t[:], acc_rs[:])

                        for cc in range(2):
                            o_t = wp.tile([128, IB], F32, name="o_t", tag="osb", bufs=3)
                            acc_cc = acc0 if cc == 0 else acc1c
                            nc.vector.tensor_mul(o_t[:], acc_cc[:], r_t[:])
                            # (o + bpt_eff) + x_residual; bpt is per-partition
                            nc.vector.scalar_tensor_tensor(
                                o_t[:], o_t[:], bpt[:, cc:cc + 1], xr[:, cc, ts(ib, IB)],
                                op0=mybir.AluOpType.add, op1=mybir.AluOpType.add,
                            )
                            nc.sync.dma_start(
                                out_d.rearrange("(o p) f -> p o f", p=128)[:, cc, ts(ib, IB)],
                                o_t[:],
                            )

    nc.compile()
    _NC_CACHE[krep] = nc
    return nc


def _prep_maps(x, Wq, bq, Wk, bk, Wv, bv, Wpt, bpt, gamma):
    bf16 = ml_dtypes.bfloat16
    f32 = np.float32
    g = float(np.asarray(gamma).reshape(-1)[0])
    # wq/wk/bq replicated at column offsets 0 and 32 (S^T 2x row-packing)
    wq2 = np.zeros((C, 48), f32)
    wq2[:, 0:DQ] = Wq.T
    wq2[:, 32:32 + DQ] = Wq.T
    wk2 = np.zeros((C, 48), f32)
    wk2[:, 0:DQ] = Wk.T
    wk2[:, 32:32 + DQ] = Wk.T
    bq_col = np.zeros((48, 1), f32)
    bq_col[0:DQ, 0] = bq
    bq_col[32:32 + DQ, 0] = bq
    # wcat_r = (g * Wpt[:, r-block] @ Wv).T, layout [c', c]
    wpt_g = (g * Wpt).astype(f32)
    wcat1 = (wpt_g[:, :C] @ Wv).T.astype(f32)
    wcat2 = (wpt_g[:, C:] @ Wv).T.astype(f32)
    bpt_eff = (g * (bpt + Wpt @ np.concatenate([bv, bv]))).astype(f32)
    bpt_col = np.ascontiguousarray(bpt_eff.reshape(2, 128).T)

    xf = np.asarray(x, f32).reshape(B, 2, C, HW)
    xb = xf.astype(bf16)
    wk2b = wk2.astype(bf16)
    w1b, w2b = wcat1.astype(bf16), wcat2.astype(bf16)
    in_maps = []
    for core in range(8):
        b, s, h = core >> 2, (core >> 1) & 1, core & 1
        # joint (x1,x2)/(w1,w2) swap for s=1: M invariant, x1b = own stream
        in_maps.append(
            dict(
                x1b=np.ascontiguousarray(xb[b, s]),
                x2b=np.ascontiguousarray(xb[b, 1 - s]),
                xres=np.ascontiguousarray(xf[b, s, :, h * HALF:(h + 1) * HALF]),
                wq2=wq2, wk2=wk2b,
                w1=(w1b if s == 0 else w2b),
                w2=(w2b if s == 0 else w1b),
                bq_col=bq_col, bpt_col=bpt_col,
            )
        )
    return in_maps


def kernel(x, Wq, bq, Wk, bk, Wv, bv, Wpt, bpt, gamma, _trace=False):
    from concourse.bass_utils import run_bass_kernel_spmd

    nc = build_bass()
    in_maps = _prep_maps(x, Wq, bq, Wk, bk, Wv, bv, Wpt, bpt, gamma)
    res = run_bass_kernel_spmd(nc, in_maps, list(range(8)), trace=_trace)

    out = np.empty((B, 2, C, HW), np.float32)
    for core in range(8):
        b, s, h = core >> 2, (core >> 1) & 1, core & 1
        out[b, s, :, h * HALF:(h + 1) * HALF] = res.results[core]["out"]
    full = out.reshape(B, 2 * C, 64, 64)
    if _trace:
        return full, res
    return full


# revision 8
# speedup vs baseline: 2.1709x; 1.3510x over previous
"""Trainium2 Bass kernel for nn_Cross_SelfAttention (B=2, C=256, H=W=64, DQ=16).

Sharding: 8 cores = (batch b) x (attn stream s) x (query half h).

Algebraic restructure: the output 1x1 conv is linear, so
  Wpt @ [attn@v1; attn@v2] = (Wpt1@V1 + Wpt2@V2) @ attn^T = M @ attn^T.
M = [C, HW] folds Wv, Wpt, gamma and the 512->256 projection into ONE
256-channel attention apply. M^T is quantized to fp8-e4m3 and the
attention apply runs as DoubleRow fp8 matmuls: one matmul contracts 256
j-positions (a pair of j-chunks), halving PE streaming time again.

fp8 range control: softmax is invariant to a per-query shift of S, so the
host computes m_i ~= max_j S[i, j] and the kernel folds it in as a 17th
contraction dim of the S^T matmul (q16 = -m_i via DMA, k16 = +1 via the
copy bias).  E' = exp(S - m_i + 5) then spans [~0, e^5] - comfortably
inside e4m3.  The shift cancels exactly in acc/rowsum.

Per core:
    M^T[j, c] = x1^T @ wcat1 + x2^T @ wcat2   (bf16 MMs, fp8 output)
    k = Wk @ x_own, q = Wq @ x_own[:, half] + bq   (bf16)
    S'[j, i] = k[:, j].q[:, i] - m_i   (pairs of j-chunks, 2x row packing)
    E' = exp(S' + 5)  (one ACT per 2-bank pair, fp8-e4m3 out)
    acc[c, i] += M^T_pair^T @ E'   (DoubleRow, K=256)
    rowsum    += ones^T @ E'       (DoubleRow, broadcast over partitions)
    out = acc * recip_fast(rowsum) + bpt_eff + x_residual(bf16)
bv is folded into bpt_eff on the host (normalization makes the missing
V-bias contribution exactly Wpt @ [bv; bv]); gamma into wcat/bpt. For
s=1 cores the host swaps (x1b,x2b) AND (w1,w2) jointly - M is invariant
and x1b is always the core's own attention stream.

Each core writes a disjoint [256, 2048] slice of the output; no
collectives needed.
"""

import os

import numpy as np
import ml_dtypes

import concourse.bass as bass
import concourse.bacc as bacc
import concourse.mybir as mybir
from concourse.tile import TileContext
from concourse.bass import ts

BF16 = mybir.dt.bfloat16
F32 = mybir.dt.float32
FP8 = mybir.dt.float8e4

B, C, HW, DQ = 2, 256, 4096, 16
HALF = HW // 2          # query positions per core
IB = 512                # i-block size (one PSUM bank at fp32)
N_IB = HALF // IB       # 4 i-blocks
N_JC = HW // 128        # 32 j-chunks
NP = N_JC // 2          # 16 j-chunk pairs per i-block
EBIAS = 5.0             # E' = exp(S - m_i + EBIAS), max ~e^5 << e4m3 max 448

_NC_CACHE = {}

KREP = int(os.environ.get("KREP", "1"))


def build_bass(krep=None):
    krep = KREP if krep is None else krep
    if krep in _NC_CACHE:
        return _NC_CACHE[krep]

    nc = bacc.Bacc("TRN2", target_bir_lowering=False, debug=False, num_devices=8)

    # Per-core inputs.
    x1_d = nc.dram_tensor("x1b", [C, HW], BF16, kind="ExternalInput")
    x2_d = nc.dram_tensor("x2b", [C, HW], BF16, kind="ExternalInput")
    xq_d = nc.dram_tensor("xq", [C, HALF], BF16, kind="ExternalInput")
    m_d = nc.dram_tensor("mrow", [1, HALF], BF16, kind="ExternalInput")
    # packed weights: [wq2(49) | wk2(49) | wcat1(256) | wcat2(256)] = 610 cols
    wp_d = nc.dram_tensor("wpack", [C, 610], BF16, kind="ExternalInput")
    bq_d = nc.dram_tensor("bq_col", [49, 1], F32, kind="ExternalInput")
    kb_d = nc.dram_tensor("kb_col", [49, 1], F32, kind="ExternalInput")
    bpt_d = nc.dram_tensor("bpt_col", [128, 2], F32, kind="ExternalInput")
    out_d = nc.dram_tensor("out", [C, HALF], F32, kind="ExternalOutput")

    with TileContext(nc) as tc:
        with (
            tc.tile_pool(name="persist", bufs=1) as pp,
            tc.tile_pool(name="work", bufs=1) as wp,
            tc.tile_pool(name="psum", bufs=1, space="PSUM") as psp,
        ):
            # ---- persistent SBUF tensors ----
            x1 = pp.tile([128, 2, HW], BF16, name="x1_sb")
            x2 = pp.tile([128, 2, HW], BF16, name="x2_sb")
            xq = pp.tile([128, 2, HALF], BF16, name="xq_sb")
            # M^T in fp8, DoubleRow layout: (j_lane, pair, ko=chunk parity, c)
            mT8 = pp.tile([128, NP, 2, C], FP8, name="mT8_sb")
            wpk = pp.tile([128, 2, 610], BF16, name="wpk_sb")
            bq = pp.tile([49, 1], F32, name="bq_sb")
            kb = pp.tile([49, 1], F32, name="kb_sb")
            bpt = pp.tile([128, 2], F32, name="bpt_sb")
            ones_dr = pp.tile([128, 2, 128], FP8, name="ones_dr")
            ebias = pp.tile([128, 1], F32, name="ebias_sb")
            qsb = pp.tile([49, HALF], BF16, name="qsb")
            ksb = pp.tile([49, HW], BF16, name="ksb")

            nc.vector.memset(ones_dr[:], 1.0)
            nc.vector.memset(ebias[:], EBIAS)

            wq = wpk[:, :, 0:49]
            wk = wpk[:, :, 49:98]
            w1s = wpk[:, :, 98:98 + C]
            w2s = wpk[:, :, 98 + C:98 + 2 * C]

            r128 = lambda ap: ap.rearrange("(o p) f -> p o f", p=128)
            nc.sync.dma_start(wpk[:], r128(wp_d))
            nc.sync.dma_start(bq[:], bq_d[:])
            nc.sync.dma_start(kb[:], kb_d[:])
            nc.sync.dma_start(bpt[:], bpt_d[:])
            for _rep in range(krep):
                # DMA order == consumption order: x1 (K proj), xq+m (Q proj),
                # x2 (M^T), 2 column-chunks each for overlap.
                for q2 in range(2):
                    nc.sync.dma_start(
                        x1[:, :, ts(q2, 2048)], r128(x1_d)[:, :, ts(q2, 2048)]
                    )
                nc.sync.dma_start(xq[:], r128(xq_d))
                # -m_i into the 17th q row of both packing replicas
                nc.sync.dma_start(qsb[16:17, :], m_d[:])
                nc.sync.dma_start(qsb[48:49, :], m_d[:])
                for q2 in range(2):
                    nc.sync.dma_start(
                        x2[:, :, ts(q2, 2048)], r128(x2_d)[:, :, ts(q2, 2048)]
                    )

                # ---- K projection (bk cancels in softmax); the copy's bias
                # writes k16 = +1 into rows 16/48.
                for p4 in range(4):
                    k_ps = psp.tile([128, 2, IB], F32, name="k_ps", tag="s", bufs=2)
                    for hf in range(2):
                        p8 = 2 * p4 + hf
                        nc.tensor.matmul(
                            k_ps[:49, hf], wk[:, 0], x1[:, 0, ts(p8, IB)],
                            start=True, stop=False,
                        )
                        nc.tensor.matmul(
                            k_ps[:49, hf], wk[:, 1], x1[:, 1, ts(p8, IB)],
                            start=False, stop=True,
                        )
                        nc.vector.tensor_scalar_add(
                            ksb[:, ts(p8, IB)], k_ps[:49, hf], kb[:]
                        )

                # ---- Q projection from xq (own i-half), bias on DVE; rows
                # 16/48 hold -m_i (DMA above), so only 0:16 / 32:48 written.
                for p2 in range(2):
                    q_ps = psp.tile([128, 2, IB], F32, name="q_ps", tag="s", bufs=2)
                    for hf in range(2):
                        p4 = 2 * p2 + hf
                        nc.tensor.matmul(
                            q_ps[:49, hf], wq[:, 0], xq[:, 0, ts(p4, IB)],
                            start=True, stop=False,
                        )
                        nc.tensor.matmul(
                            q_ps[:49, hf], wq[:, 1], xq[:, 1, ts(p4, IB)],
                            start=False, stop=True,
                        )
                        nc.vector.tensor_scalar_add(
                            qsb[0:16, ts(p4, IB)], q_ps[0:16, hf], bq[0:16]
                        )
                        nc.vector.tensor_scalar_add(
                            qsb[32:48, ts(p4, IB)], q_ps[32:48, hf], bq[32:48]
                        )

                # ---- M^T = x1^T @ wcat1 + x2^T @ wcat2, fp8 out ----
                for jc in range(N_JC):
                    m_ps = psp.tile([128, IB], F32, name="m_ps", tag="acc", bufs=4)
                    for cp in range(4):
                        xs_ = x1 if cp < 2 else x2
                        ws_ = w1s if cp < 2 else w2s
                        o = cp % 2
                        nc.tensor.matmul(
                            m_ps[:, 0:C], xs_[:, o, ts(jc, 128)], ws_[:, o],
                            start=(cp == 0), stop=(cp == 3),
                        )
                    nc.vector.tensor_copy(mT8[:, jc // 2, jc % 2, :], m_ps[:, 0:C])

                # ---- main attention loop: software-pipelined pairs ----
                def issue_st(g):
                    ib, p = divmod(g, NP)
                    s_p = psp.tile([128, 2, IB], F32, name="s_p", tag="s", bufs=2)
                    nc.tensor.matmul(
                        s_p[:, 0], ksb[0:17, ts(2 * p, 128)],
                        qsb[0:17, ts(ib, IB)],
                        start=True, stop=True, tile_position=(0, 0),
                    )
                    nc.tensor.matmul(
                        s_p[:, 1], ksb[32:49, ts(2 * p + 1, 128)],
                        qsb[32:49, ts(ib, IB)],
                        start=True, stop=True, tile_position=(32, 0),
                    )
                    return s_p

                s_cur = issue_st(0)
                for g in range(N_IB * NP):
                    ib, p = divmod(g, NP)
                    if p == 0:
                        # acc_rs first: its slot frees earliest (at recip)
                        acc_rs = psp.tile([128, IB], F32, name="acc_rs", tag="acc", bufs=4)
                        acc0 = psp.tile([128, IB], F32, name="acc0", tag="acc", bufs=4)
                        acc1c = psp.tile([128, IB], F32, name="acc1c", tag="acc", bufs=4)
                    e_p = wp.tile([128, 2, IB], FP8, name="e_p", tag="E", bufs=3)
                    nc.scalar.activation(
                        e_p[:], s_cur[:], mybir.ActivationFunctionType.Exp,
                        bias=ebias[:],
                    )
                    if g + 1 < N_IB * NP:
                        s_cur = issue_st(g + 1)
                    nc.tensor.matmul(
                        acc0[:], mT8[:, p, :, 0:128], e_p[:],
                        start=(p == 0), stop=(p == NP - 1),
                        perf_mode=mybir.MatmulPerfMode.DoubleRow,
                    )
                    nc.tensor.matmul(
                        acc1c[:], mT8[:, p, :, 128:256], e_p[:],
                        start=(p == 0), stop=(p == NP - 1),
                        perf_mode=mybir.MatmulPerfMode.DoubleRow,
                    )
                    nc.tensor.matmul(
                        acc_rs[:], ones_dr[:], e_p[:],
                        start=(p == 0), stop=(p == NP - 1),
                        perf_mode=mybir.MatmulPerfMode.DoubleRow,
                    )

                    if p == NP - 1:
                        r_t = wp.tile([128, IB], F32, name="r_t", tag="R", bufs=2)
                        nc.vector.reciprocal_approx_fast(r_t[:], acc_rs[:])
                        for cc in range(2):
                            o_t = wp.tile([128, IB], F32, name="o_t", tag="osb", bufs=3)
                            acc_cc = acc0 if cc == 0 else acc1c
                            nc.vector.tensor_mul(o_t[:], acc_cc[:], r_t[:])
                            # (o + bpt_eff) + x_residual; bpt is per-partition
                            nc.vector.scalar_tensor_tensor(
                                o_t[:], o_t[:], bpt[:, cc:cc + 1], xq[:, cc, ts(ib, IB)],
                                op0=mybir.AluOpType.add, op1=mybir.AluOpType.add,
                            )
                            nc.sync.dma_start(
                                out_d.rearrange("(o p) f -> p o f", p=128)[:, cc, ts(ib, IB)],
                                o_t[:],
                            )

    nc.compile()
    _NC_CACHE[krep] = nc
    return nc


def _prep_maps(x, Wq, bq, Wk, bk, Wv, bv, Wpt, bpt, gamma):
    bf16 = ml_dtypes.bfloat16
    f32 = np.float32
    g = float(np.asarray(gamma).reshape(-1)[0])
    # wq/wk replicated at column offsets 0 and 32 (S^T 2x row-packing);
    # col 16/48 zero (the shift dim, filled on-device).
    wq2 = np.zeros((C, 49), f32)
    wq2[:, 0:DQ] = Wq.T
    wq2[:, 32:32 + DQ] = Wq.T
    wk2 = np.zeros((C, 49), f32)
    wk2[:, 0:DQ] = Wk.T
    wk2[:, 32:32 + DQ] = Wk.T
    bq_col = np.zeros((49, 1), f32)
    bq_col[0:DQ, 0] = bq
    bq_col[32:32 + DQ, 0] = bq
    kb_col = np.zeros((49, 1), f32)
    kb_col[16, 0] = 1.0
    kb_col[48, 0] = 1.0
    # wcat_r = (g * Wpt[:, r-block] @ Wv).T, layout [c', c]
    wpt_g = (g * Wpt).astype(f32)
    wcat1 = (wpt_g[:, :C] @ Wv).T.astype(f32)
    wcat2 = (wpt_g[:, C:] @ Wv).T.astype(f32)
    bpt_eff = (g * (bpt + Wpt @ np.concatenate([bv, bv]))).astype(f32)
    bpt_col = np.ascontiguousarray(bpt_eff.reshape(2, 128).T)

    xf = np.asarray(x, f32).reshape(B, 2, C, HW)
    xb = xf.astype(bf16)
    wpack1 = np.ascontiguousarray(
        np.concatenate([wq2, wk2, wcat1, wcat2], axis=1).astype(bf16))
    wpack2 = np.ascontiguousarray(
        np.concatenate([wq2, wk2, wcat2, wcat1], axis=1).astype(bf16))

    # per-query S rowmax (fp32, shared by the two query-half cores of (b,s));
    # any value near the true rowmax works - it only conditions fp8 range.
    mrow = np.empty((B, 2, HW), f32)
    for b in range(B):
        for s in range(2):
            q = Wq @ xf[b, s] + bq.reshape(-1, 1)
            k = Wk @ xf[b, s]
            mrow[b, s] = (q.T @ k).max(axis=1)

    in_maps = []
    for core in range(8):
        b, s, h = core >> 2, (core >> 1) & 1, core & 1
        # joint (x1,x2)/(w1,w2) swap for s=1: M invariant, x1b = own stream
        in_maps.append(
            dict(
                x1b=np.ascontiguousarray(xb[b, s]),
                x2b=np.ascontiguousarray(xb[b, 1 - s]),
                xq=np.ascontiguousarray(xb[b, s, :, h * HALF:(h + 1) * HALF]),
                mrow=np.ascontiguousarray(
                    (-mrow[b, s, h * HALF:(h + 1) * HALF]).astype(bf16).reshape(1, HALF)),
                wpack=(wpack1 if s == 0 else wpack2),
                bq_col=bq_col, kb_col=kb_col, bpt_col=bpt_col,
            )
        )
    return in_maps


def kernel(x, Wq, bq, Wk, bk, Wv, bv, Wpt, bpt, gamma, _trace=False):
    from concourse.bass_utils import run_bass_kernel_spmd

    nc = build_bass()
    in_maps = _prep_maps(x, Wq, bq, Wk, bk, Wv, bv, Wpt, bpt, gamma)
    res = run_bass_kernel_spmd(nc, in_maps, list(range(8)), trace=_trace)

    out = np.empty((B, 2, C, HW), np.float32)
    for core in range(8):
        b, s, h = core >> 2, (core >> 1) & 1, core & 1
        out[b, s, :, h * HALF:(h + 1) * HALF] = res.results[core]["out"]
    full = out.reshape(B, 2 * C, 64, 64)
    if _trace:
        return full, res
    return full


# revision 12
# speedup vs baseline: 2.1855x; 1.0067x over previous
"""Trainium2 Bass kernel for nn_Cross_SelfAttention (B=2, C=256, H=W=64, DQ=16).

Sharding: 8 cores = (batch b) x (attn stream s) x (query half h).

Algebraic restructure: the output 1x1 conv is linear, so
  Wpt @ [attn@v1; attn@v2] = (Wpt1@V1 + Wpt2@V2) @ attn^T = M @ attn^T.
M = [C, HW] folds Wv, Wpt, gamma and the 512->256 projection into ONE
256-channel attention apply. M^T is quantized to fp8-e4m3 and the
attention apply runs as DoubleRow fp8 matmuls: one matmul contracts 256
j-positions (a pair of j-chunks), halving PE streaming time again.

fp8 range control: softmax is invariant to a per-query shift of S, so the
host computes m_i ~= max_j S[i, j] and the kernel folds it in as a 17th
contraction dim of the S^T matmul (q16 = -m_i via DMA, k16 = +1 via the
copy bias).  E' = exp(S - m_i + 5) then spans [~0, e^5] - comfortably
inside e4m3.  The shift cancels exactly in acc/rowsum.

Per core:
    M^T[j, c] = x1^T @ wcat1 + x2^T @ wcat2   (bf16 MMs, fp8 output)
    k = Wk @ x_own, q = Wq @ x_own[:, half] + bq   (bf16)
    S'[j, i] = k[:, j].q[:, i] - m_i   (pairs of j-chunks, 2x row packing)
    E' = exp(S' + 5)  (one ACT per 2-bank pair, fp8-e4m3 out)
    acc[c, i] += M^T_pair^T @ E'   (DoubleRow, K=256)
    rowsum    += ones^T @ E'       (DoubleRow, broadcast over partitions)
    out = acc * recip_fast(rowsum) + bpt_eff + x_residual(bf16)
bv is folded into bpt_eff on the host (normalization makes the missing
V-bias contribution exactly Wpt @ [bv; bv]); gamma into wcat/bpt. For
s=1 cores the host swaps (x1b,x2b) AND (w1,w2) jointly - M is invariant
and x1b is always the core's own attention stream.

Each core writes a disjoint [256, 2048] slice of the output; no
collectives needed.
"""

import os

import numpy as np
import ml_dtypes

import concourse.bass as bass
import concourse.bacc as bacc
import concourse.mybir as mybir
from concourse.tile import TileContext
from concourse.bass import ts

BF16 = mybir.dt.bfloat16
F32 = mybir.dt.float32
FP8 = mybir.dt.float8e4

B, C, HW, DQ = 2, 256, 4096, 16
HALF = HW // 2          # query positions per core
IB = 512                # i-block size (one PSUM bank at fp32)
N_IB = HALF // IB       # 4 i-blocks
N_JC = HW // 128        # 32 j-chunks
NP = N_JC // 2          # 16 j-chunk pairs per i-block
EBIAS = 5.0             # E' = exp(S - m_i + EBIAS), max ~e^5 << e4m3 max 448

_NC_CACHE = {}

KREP = int(os.environ.get("KREP", "1"))


def build_bass(krep=None):
    krep = KREP if krep is None else krep
    if krep in _NC_CACHE:
        return _NC_CACHE[krep]

    nc = bacc.Bacc("TRN2", target_bir_lowering=False, debug=False, num_devices=8)

    # Per-core inputs.
    x1_d = nc.dram_tensor("x1b", [C, HW], BF16, kind="ExternalInput")
    x2_d = nc.dram_tensor("x2b", [C, HW], BF16, kind="ExternalInput")
    xq_d = nc.dram_tensor("xq", [C, HALF], BF16, kind="ExternalInput")
    m_d = nc.dram_tensor("mrow", [1, HALF], BF16, kind="ExternalInput")
    # packed weights: [wq2(49) | wk2(49) | wcat1(256) | wcat2(256)] = 610 cols,
    # pre-interleaved on host to [128 partitions, 2*610] for 1-descriptor rows
    wp_d = nc.dram_tensor("wpack", [128, 2, 610], BF16, kind="ExternalInput")
    bq_d = nc.dram_tensor("bq_col", [49, 1], F32, kind="ExternalInput")
    kb_d = nc.dram_tensor("kb_col", [49, 1], F32, kind="ExternalInput")
    bpt_d = nc.dram_tensor("bpt_col", [128, 2], F32, kind="ExternalInput")
    out_d = nc.dram_tensor("out", [C, HALF], F32, kind="ExternalOutput")

    with TileContext(nc) as tc:
        with (
            tc.tile_pool(name="persist", bufs=1) as pp,
            tc.tile_pool(name="work", bufs=1) as wp,
            tc.tile_pool(name="psum", bufs=1, space="PSUM") as psp,
        ):
            # ---- persistent SBUF tensors ----
            x1 = pp.tile([128, 2, HW], BF16, name="x1_sb")
            x2 = pp.tile([128, 2, HW], BF16, name="x2_sb")
            xq = pp.tile([128, 2, HALF], BF16, name="xq_sb")
            # M^T in fp8, DoubleRow layout: (j_lane, pair, ko=chunk parity, c)
            mT8 = pp.tile([128, NP, 2, C], FP8, name="mT8_sb")
            wpk = pp.tile([128, 2, 610], BF16, name="wpk_sb")
            bq = pp.tile([49, 1], F32, name="bq_sb")
            kb = pp.tile([49, 1], F32, name="kb_sb")
            bpt = pp.tile([128, 2], F32, name="bpt_sb")
            ones_dr = pp.tile([128, 2, 128], FP8, name="ones_dr")
            ebias = pp.tile([128, 1], F32, name="ebias_sb")
            qsb = pp.tile([49, HALF], BF16, name="qsb")
            ksb = pp.tile([49, HW], BF16, name="ksb")

            nc.vector.memset(ones_dr[:], 1.0)
            nc.vector.memset(ebias[:], EBIAS)

            wq = wpk[:, :, 0:49]
            wk = wpk[:, :, 49:98]
            w1s = wpk[:, :, 98:98 + C]
            w2s = wpk[:, :, 98 + C:98 + 2 * C]

            r128 = lambda ap: ap.rearrange("(o p) f -> p o f", p=128)
            nc.sync.dma_start(bq[:], bq_d[:])
            nc.sync.dma_start(kb[:], kb_d[:])
            nc.sync.dma_start(bpt[:], bpt_d[:])
            for _rep in range(krep):
                # DMA order == consumption order: x1c0 (K proj 0-3), weights,
                # x2c0 (M^T 0-15), x1c1 (K 4-7), x2c1 (M^T 16-31), xq+m (Q).
                nc.sync.dma_start(x1[:, :, ts(0, 2048)], r128(x1_d)[:, :, ts(0, 2048)])
                nc.sync.dma_start(wpk[:], wp_d[:])
                nc.sync.dma_start(x2[:, :, ts(0, 2048)], r128(x2_d)[:, :, ts(0, 2048)])
                nc.sync.dma_start(x1[:, :, ts(1, 2048)], r128(x1_d)[:, :, ts(1, 2048)])
                nc.sync.dma_start(x2[:, :, ts(1, 2048)], r128(x2_d)[:, :, ts(1, 2048)])
                nc.sync.dma_start(xq[:], r128(xq_d))
                # -m_i into the 17th q row of both packing replicas
                nc.sync.dma_start(qsb[16:17, :], m_d[:])
                nc.sync.dma_start(qsb[48:49, :], m_d[:])

                def k_proj(p4):
                    # the copy's bias writes k16 = +1 into rows 16/48
                    k_ps = psp.tile([128, 2, IB], F32, name="k_ps", tag="s", bufs=2)
                    for hf in range(2):
                        p8 = 2 * p4 + hf
                        nc.tensor.matmul(
                            k_ps[:49, hf], wk[:, 0], x1[:, 0, ts(p8, IB)],
                            start=True, stop=False,
                        )
                        nc.tensor.matmul(
                            k_ps[:49, hf], wk[:, 1], x1[:, 1, ts(p8, IB)],
                            start=False, stop=True,
                        )
                        nc.vector.tensor_scalar_add(
                            ksb[:, ts(p8, IB)], k_ps[:49, hf], kb[:]
                        )

                def m_chunk(jc):
                    m_ps = psp.tile([128, IB], F32, name="m_ps", tag="acc", bufs=4)
                    for cp in range(4):
                        xs_ = x1 if cp < 2 else x2
                        ws_ = w1s if cp < 2 else w2s
                        o = cp % 2
                        nc.tensor.matmul(
                            m_ps[:, 0:C], xs_[:, o, ts(jc, 128)], ws_[:, o],
                            start=(cp == 0), stop=(cp == 3),
                        )
                    nc.vector.tensor_copy(mT8[:, jc // 2, jc % 2, :], m_ps[:, 0:C])

                # PE order follows DMA arrival order
                for p4 in range(2):
                    k_proj(p4)
                for jc in range(16):
                    m_chunk(jc)
                for p4 in range(2, 4):
                    k_proj(p4)
                for jc in range(16, N_JC):
                    m_chunk(jc)

                # ---- Q projection from xq (own i-half), bias on DVE; rows
                # 16/48 hold -m_i (DMA above), so only 0:16 / 32:48 written.
                for p2 in range(2):
                    q_ps = psp.tile([128, 2, IB], F32, name="q_ps", tag="s", bufs=2)
                    for hf in range(2):
                        p4 = 2 * p2 + hf
                        nc.tensor.matmul(
                            q_ps[:49, hf], wq[:, 0], xq[:, 0, ts(p4, IB)],
                            start=True, stop=False,
                        )
                        nc.tensor.matmul(
                            q_ps[:49, hf], wq[:, 1], xq[:, 1, ts(p4, IB)],
                            start=False, stop=True,
                        )
                        nc.vector.tensor_scalar_add(
                            qsb[0:16, ts(p4, IB)], q_ps[0:16, hf], bq[0:16]
                        )
                        nc.vector.tensor_scalar_add(
                            qsb[32:48, ts(p4, IB)], q_ps[32:48, hf], bq[32:48]
                        )

                # ---- main attention loop: software-pipelined pairs ----
                def issue_st(g):
                    ib, p = divmod(g, NP)
                    s_p = psp.tile([128, 2, IB], F32, name="s_p", tag="s", bufs=2)
                    nc.tensor.matmul(
                        s_p[:, 0], ksb[0:17, ts(2 * p, 128)],
                        qsb[0:17, ts(ib, IB)],
                        start=True, stop=True, tile_position=(0, 0),
                    )
                    nc.tensor.matmul(
                        s_p[:, 1], ksb[32:49, ts(2 * p + 1, 128)],
                        qsb[32:49, ts(ib, IB)],
                        start=True, stop=True, tile_position=(32, 0),
                    )
                    return s_p

                s_cur = issue_st(0)
                for g in range(N_IB * NP):
                    ib, p = divmod(g, NP)
                    if p == 0:
                        # acc_rs first: its slot frees earliest (at recip)
                        acc_rs = psp.tile([128, IB], F32, name="acc_rs", tag="acc", bufs=4)
                        acc0 = psp.tile([128, IB], F32, name="acc0", tag="acc", bufs=4)
                        acc1c = psp.tile([128, IB], F32, name="acc1c", tag="acc", bufs=4)
                    e_p = wp.tile([128, 2, IB], FP8, name="e_p", tag="E", bufs=3)
                    nc.scalar.activation(
                        e_p[:], s_cur[:], mybir.ActivationFunctionType.Exp,
                        bias=ebias[:],
                    )
                    if g + 1 < N_IB * NP:
                        s_cur = issue_st(g + 1)
                    nc.tensor.matmul(
                        acc0[:], mT8[:, p, :, 0:128], e_p[:],
                        start=(p == 0), stop=(p == NP - 1),
                        perf_mode=mybir.MatmulPerfMode.DoubleRow,
                    )
                    nc.tensor.matmul(
                        acc1c[:], mT8[:, p, :, 128:256], e_p[:],
                        start=(p == 0), stop=(p == NP - 1),
                        perf_mode=mybir.MatmulPerfMode.DoubleRow,
                    )
                    nc.tensor.matmul(
                        acc_rs[:], ones_dr[:], e_p[:],
                        start=(p == 0), stop=(p == NP - 1),
                        perf_mode=mybir.MatmulPerfMode.DoubleRow,
                    )

                    if p == NP - 1:
                        # Last i-block: halve the serial recip->mul->add->DMA
                        # tail by processing two 256-column halves.
                        nh = 2 if ib == N_IB - 1 else 1
                        hw_ = IB // nh
                        out_r = out_d.rearrange("(o p) f -> p o f", p=128)
                        for hh in range(nh):
                            sl = slice(hh * hw_, (hh + 1) * hw_)
                            r_t = wp.tile([128, IB], F32, name="r_t", tag="R", bufs=2)
                            nc.vector.reciprocal_approx_fast(r_t[:, sl], acc_rs[:, sl])
                            o_ts = []
                            for cc in range(2):
                                o_t = wp.tile([128, IB], F32, name="o_t", tag="osb", bufs=3)
                                o_ts.append(o_t)
                                acc_cc = acc0 if cc == 0 else acc1c
                                nc.vector.tensor_mul(o_t[:, sl], acc_cc[:, sl], r_t[:, sl])
                            for cc in range(2):
                                # (o + bpt_eff) + x_residual; bpt per-partition
                                o_t = o_ts[cc]
                                nc.vector.scalar_tensor_tensor(
                                    o_t[:, sl], o_t[:, sl], bpt[:, cc:cc + 1],
                                    xq[:, cc, ib * IB + hh * hw_:ib * IB + (hh + 1) * hw_],
                                    op0=mybir.AluOpType.add, op1=mybir.AluOpType.add,
                                )
                                nc.sync.dma_start(
                                    out_r[:, cc, ib * IB + hh * hw_:ib * IB + (hh + 1) * hw_],
                                    o_t[:, sl],
                                )

    nc.compile()
    _NC_CACHE[krep] = nc
    return nc


def _prep_maps(x, Wq, bq, Wk, bk, Wv, bv, Wpt, bpt, gamma):
    bf16 = ml_dtypes.bfloat16
    f32 = np.float32
    g = float(np.asarray(gamma).reshape(-1)[0])
    # wq/wk replicated at column offsets 0 and 32 (S^T 2x row-packing);
    # col 16/48 zero (the shift dim, filled on-device).
    wq2 = np.zeros((C, 49), f32)
    wq2[:, 0:DQ] = Wq.T
    wq2[:, 32:32 + DQ] = Wq.T
    wk2 = np.zeros((C, 49), f32)
    wk2[:, 0:DQ] = Wk.T
    wk2[:, 32:32 + DQ] = Wk.T
    bq_col = np.zeros((49, 1), f32)
    bq_col[0:DQ, 0] = bq
    bq_col[32:32 + DQ, 0] = bq
    kb_col = np.zeros((49, 1), f32)
    kb_col[16, 0] = 1.0
    kb_col[48, 0] = 1.0
    # wcat_r = (g * Wpt[:, r-block] @ Wv).T, layout [c', c]
    wpt_g = (g * Wpt).astype(f32)
    wcat1 = (wpt_g[:, :C] @ Wv).T.astype(f32)
    wcat2 = (wpt_g[:, C:] @ Wv).T.astype(f32)
    bpt_eff = (g * (bpt + Wpt @ np.concatenate([bv, bv]))).astype(f32)
    bpt_col = np.ascontiguousarray(bpt_eff.reshape(2, 128).T)

    xf = np.asarray(x, f32).reshape(B, 2, C, HW)
    xb = xf.astype(bf16)
    def interleave(w):  # [C, F] -> [128, 2, F] partition-major (1 desc/row)
        return np.ascontiguousarray(
            w.astype(bf16).reshape(2, 128, -1).transpose(1, 0, 2))

    wpack1 = interleave(np.concatenate([wq2, wk2, wcat1, wcat2], axis=1))
    wpack2 = interleave(np.concatenate([wq2, wk2, wcat2, wcat1], axis=1))

    # per-query S rowmax (fp32, shared by the two query-half cores of (b,s));
    # any value near the true rowmax works - it only conditions fp8 range.
    mrow = np.empty((B, 2, HW), f32)
    for b in range(B):
        for s in range(2):
            q = Wq @ xf[b, s] + bq.reshape(-1, 1)
            k = Wk @ xf[b, s]
            mrow[b, s] = (q.T @ k).max(axis=1)

    in_maps = []
    for core in range(8):
        b, s, h = core >> 2, (core >> 1) & 1, core & 1
        # joint (x1,x2)/(w1,w2) swap for s=1: M invariant, x1b = own stream
        in_maps.append(
            dict(
                x1b=np.ascontiguousarray(xb[b, s]),
                x2b=np.ascontiguousarray(xb[b, 1 - s]),
                xq=np.ascontiguousarray(xb[b, s, :, h * HALF:(h + 1) * HALF]),
                mrow=np.ascontiguousarray(
                    (-mrow[b, s, h * HALF:(h + 1) * HALF]).astype(bf16).reshape(1, HALF)),
                wpack=(wpack1 if s == 0 else wpack2),
                bq_col=bq_col, kb_col=kb_col, bpt_col=bpt_col,
            )
        )
    return in_maps


def kernel(x, Wq, bq, Wk, bk, Wv, bv, Wpt, bpt, gamma, _trace=False):
    from concourse.bass_utils import run_bass_kernel_spmd

    nc = build_bass()
    in_maps = _prep_maps(x, Wq, bq, Wk, bk, Wv, bv, Wpt, bpt, gamma)
    res = run_bass_kernel_spmd(nc, in_maps, list(range(8)), trace=_trace)

    out = np.empty((B, 2, C, HW), np.float32)
    for core in range(8):
        b, s, h = core >> 2, (core >> 1) & 1, core & 1
        out[b, s, :, h * HALF:(h + 1) * HALF] = res.results[core]["out"]
    full = out.reshape(B, 2 * C, 64, 64)
    if _trace:
        return full, res
    return full


# revision 17
# speedup vs baseline: 2.2684x; 1.0379x over previous
"""Trainium2 Bass kernel for nn_Cross_SelfAttention (B=2, C=256, H=W=64, DQ=16).

Sharding: 8 cores = (batch b) x (attn stream s) x (query half h).

Algebraic restructure: the output 1x1 conv is linear, so
  Wpt @ [attn@v1; attn@v2] = (Wpt1@V1 + Wpt2@V2) @ attn^T = M @ attn^T.
M = [C, HW] folds Wv, Wpt, gamma and the 512->256 projection into ONE
256-channel attention apply. M^T is quantized to fp8-e4m3 and the
attention apply runs as DoubleRow fp8 matmuls: one matmul contracts 256
j-positions (a pair of j-chunks), halving PE streaming time again.

fp8 range control: softmax is invariant to a per-query shift of S, so the
host computes m_i ~= max_j S[i, j] and the kernel folds it in as a 17th
contraction dim of the S^T matmul (q16 = -m_i via DMA, k16 = +1 via the
copy bias).  E' = exp(S - m_i + 5) then spans [~0, e^5] - comfortably
inside e4m3.  The shift cancels exactly in acc/rowsum.

Per core:
    M^T[j, c] = x1^T @ wcat1 + x2^T @ wcat2   (bf16 MMs, fp8 output)
    k = Wk @ x_own, q = Wq @ x_own[:, half] + bq   (bf16)
    S'[j, i] = k[:, j].q[:, i] - m_i   (pairs of j-chunks, 2x row packing)
    E' = exp(S' + 5)  (one ACT per 2-bank pair, fp8-e4m3 out)
    acc[c, i] += M^T_pair^T @ E'   (DoubleRow, K=256)
    rowsum    += ones^T @ E'       (DoubleRow, broadcast over partitions)
    out = acc * recip_fast(rowsum) + bpt_eff + x_residual(bf16)
bv is folded into bpt_eff on the host (normalization makes the missing
V-bias contribution exactly Wpt @ [bv; bv]); gamma into wcat/bpt. For
s=1 cores the host swaps (x1b,x2b) AND (w1,w2) jointly - M is invariant
and x1b is always the core's own attention stream.

Each core writes a disjoint [256, 2048] slice of the output; no
collectives needed.
"""

import os

import numpy as np
import ml_dtypes

import concourse.bass as bass
import concourse.bacc as bacc
import concourse.mybir as mybir
from concourse.tile import TileContext
from concourse.bass import ts

BF16 = mybir.dt.bfloat16
F32 = mybir.dt.float32
FP8 = mybir.dt.float8e4

B, C, HW, DQ = 2, 256, 4096, 16
HALF = HW // 2          # query positions per core
IB = 512                # i-block size (one PSUM bank at fp32)
N_IB = HALF // IB       # 4 i-blocks
N_JC = HW // 128        # 32 j-chunks
NP = N_JC // 2          # 16 j-chunk pairs per i-block
EBIAS = 5.0             # E' = exp(S - m_i + EBIAS), max ~e^5 << e4m3 max 448

_NC_CACHE = {}

KREP = int(os.environ.get("KREP", "1"))


def build_bass(krep=None):
    krep = KREP if krep is None else krep
    if krep in _NC_CACHE:
        return _NC_CACHE[krep]

    nc = bacc.Bacc("TRN2", target_bir_lowering=False, debug=False, num_devices=8)

    # Per-core inputs.
    # x1b/x2b are column-rotated per core so the own query half sits at
    # columns 0:HALF (attention is j-permutation invariant when K, M and
    # rowsum share the order) - Q proj and the residual read x1 directly.
    x1_d = nc.dram_tensor("x1b", [C, HW], BF16, kind="ExternalInput")
    x2_d = nc.dram_tensor("x2b", [C, HW], BF16, kind="ExternalInput")
    m_d = nc.dram_tensor("mrow", [1, HALF], BF16, kind="ExternalInput")
    # packed weights: [wq2(49) | wk2(49) | wcat1(256) | wcat2(256)] = 610 cols,
    # pre-interleaved on host to [128 partitions, 2*610] for 1-descriptor rows
    wp_d = nc.dram_tensor("wpack", [128, 2, 610], BF16, kind="ExternalInput")
    bq_d = nc.dram_tensor("bq_col", [49, 1], F32, kind="ExternalInput")
    kb_d = nc.dram_tensor("kb_col", [49, 1], F32, kind="ExternalInput")
    bpt_d = nc.dram_tensor("bpt_col", [128, 2], F32, kind="ExternalInput")
    out_d = nc.dram_tensor("out", [C, HALF], F32, kind="ExternalOutput")

    with TileContext(nc) as tc:
        with (
            tc.tile_pool(name="persist", bufs=1) as pp,
            tc.tile_pool(name="work", bufs=1) as wp,
            tc.tile_pool(name="psum", bufs=1, space="PSUM") as psp,
        ):
            # ---- persistent SBUF tensors ----
            x1 = pp.tile([128, 2, HW], BF16, name="x1_sb")
            x2 = pp.tile([128, 2, HW], BF16, name="x2_sb")
            xq = x1[:, :, 0:HALF]  # own query half (rotated to the front)
            # M^T in fp8, DoubleRow layout: (j_lane, pair, ko=chunk parity, c)
            mT8 = pp.tile([128, NP, 2, C], FP8, name="mT8_sb")
            wpk = pp.tile([128, 2, 610], BF16, name="wpk_sb")
            bq = pp.tile([49, 1], F32, name="bq_sb")
            kb = pp.tile([49, 1], F32, name="kb_sb")
            bpt = pp.tile([128, 2], F32, name="bpt_sb")
            ones_dr = pp.tile([128, 2, 128], FP8, name="ones_dr")
            ebias = pp.tile([128, 1], F32, name="ebias_sb")
            qsb = pp.tile([49, HALF], BF16, name="qsb")
            ksb = pp.tile([49, HW], BF16, name="ksb")

            nc.vector.memset(ones_dr[:], 1.0)
            nc.vector.memset(ebias[:], EBIAS)

            wq = wpk[:, :, 0:49]
            wk = wpk[:, :, 49:98]
            w1s = wpk[:, :, 98:98 + C]
            w2s = wpk[:, :, 98 + C:98 + 2 * C]

            r128 = lambda ap: ap.rearrange("(o p) f -> p o f", p=128)
            nc.sync.dma_start(bq[:], bq_d[:])
            nc.sync.dma_start(kb[:], kb_d[:])
            nc.sync.dma_start(bpt[:], bpt_d[:])
            for _rep in range(krep):
                # DMA order == consumption order: x1c0 (Q + K proj 0-3),
                # weights, x2c0 (M^T 0-15), x1c1 (K 4-7), x2c1 (M^T 16-31).
                nc.sync.dma_start(x1[:, :, ts(0, 2048)], r128(x1_d)[:, :, ts(0, 2048)])
                nc.sync.dma_start(wpk[:], wp_d[:])
                # -m_i into the 17th q row of both packing replicas
                nc.sync.dma_start(qsb[16:17, :], m_d[:])
                nc.sync.dma_start(qsb[48:49, :], m_d[:])
                nc.sync.dma_start(x2[:, :, ts(0, 2048)], r128(x2_d)[:, :, ts(0, 2048)])
                nc.sync.dma_start(x1[:, :, ts(1, 2048)], r128(x1_d)[:, :, ts(1, 2048)])
                nc.sync.dma_start(x2[:, :, ts(1, 2048)], r128(x2_d)[:, :, ts(1, 2048)])

                def k_proj(p4):
                    # the copy's bias writes k16 = +1 into rows 16/48
                    k_ps = psp.tile([128, 2, IB], F32, name="k_ps", tag="s", bufs=2)
                    for hf in range(2):
                        p8 = 2 * p4 + hf
                        nc.tensor.matmul(
                            k_ps[:49, hf], wk[:, 0], x1[:, 0, ts(p8, IB)],
                            start=True, stop=False,
                        )
                        nc.tensor.matmul(
                            k_ps[:49, hf], wk[:, 1], x1[:, 1, ts(p8, IB)],
                            start=False, stop=True,
                        )
                        nc.vector.tensor_scalar_add(
                            ksb[:, ts(p8, IB)], k_ps[:49, hf], kb[:]
                        )

                def m_chunk(jc):
                    m_ps = psp.tile([128, IB], F32, name="m_ps", tag="acc", bufs=4)
                    for cp in range(4):
                        xs_ = x1 if cp < 2 else x2
                        ws_ = w1s if cp < 2 else w2s
                        o = cp % 2
                        nc.tensor.matmul(
                            m_ps[:, 0:C], xs_[:, o, ts(jc, 128)], ws_[:, o],
                            start=(cp == 0), stop=(cp == 3),
                        )
                    nc.vector.tensor_copy(mT8[:, jc // 2, jc % 2, :], m_ps[:, 0:C])

                def q_proj(p2):
                    # bias on DVE; rows 16/48 hold -m_i (DMA above), so only
                    # 0:16 / 32:48 are written.
                    q_ps = psp.tile([128, 2, IB], F32, name="q_ps", tag="s", bufs=2)
                    for hf in range(2):
                        p4 = 2 * p2 + hf
                        nc.tensor.matmul(
                            q_ps[:49, hf], wq[:, 0], xq[:, 0, ts(p4, IB)],
                            start=True, stop=False,
                        )
                        nc.tensor.matmul(
                            q_ps[:49, hf], wq[:, 1], xq[:, 1, ts(p4, IB)],
                            start=False, stop=True,
                        )
                        nc.vector.tensor_scalar_add(
                            qsb[0:16, ts(p4, IB)], q_ps[0:16, hf], bq[0:16]
                        )
                        nc.vector.tensor_scalar_add(
                            qsb[32:48, ts(p4, IB)], q_ps[32:48, hf], bq[32:48]
                        )

                # PE order follows DMA arrival order
                for p2 in range(2):
                    q_proj(p2)
                for p4 in range(2):
                    k_proj(p4)
                for jc in range(16):
                    m_chunk(jc)
                for p4 in range(2, 4):
                    k_proj(p4)
                for jc in range(16, N_JC):
                    m_chunk(jc)

                # ---- main attention loop: software-pipelined pairs ----
                def issue_st(g):
                    ib, p = divmod(g, NP)
                    s_p = psp.tile([128, 2, IB], F32, name="s_p", tag="s", bufs=2)
                    nc.tensor.matmul(
                        s_p[:, 0], ksb[0:17, ts(2 * p, 128)],
                        qsb[0:17, ts(ib, IB)],
                        start=True, stop=True, tile_position=(0, 0),
                    )
                    nc.tensor.matmul(
                        s_p[:, 1], ksb[32:49, ts(2 * p + 1, 128)],
                        qsb[32:49, ts(ib, IB)],
                        start=True, stop=True, tile_position=(32, 0),
                    )
                    return s_p

                s_cur = issue_st(0)
                for g in range(N_IB * NP):
                    ib, p = divmod(g, NP)
                    if p == 0:
                        # acc_rs first: its slot frees earliest (at recip)
                        acc_rs = psp.tile([128, IB], F32, name="acc_rs", tag="acc", bufs=4)
                        acc0 = psp.tile([128, IB], F32, name="acc0", tag="acc", bufs=4)
                        acc1c = psp.tile([128, IB], F32, name="acc1c", tag="acc", bufs=4)
                    e_p = wp.tile([128, 2, IB], FP8, name="e_p", tag="E", bufs=3)
                    nc.scalar.activation(
                        e_p[:], s_cur[:], mybir.ActivationFunctionType.Exp,
                        bias=ebias[:],
                    )
                    if g + 1 < N_IB * NP:
                        s_cur = issue_st(g + 1)
                    nc.tensor.matmul(
                        acc0[:], mT8[:, p, :, 0:128], e_p[:],
                        start=(p == 0), stop=(p == NP - 1),
                        perf_mode=mybir.MatmulPerfMode.DoubleRow,
                    )
                    nc.tensor.matmul(
                        acc1c[:], mT8[:, p, :, 128:256], e_p[:],
                        start=(p == 0), stop=(p == NP - 1),
                        perf_mode=mybir.MatmulPerfMode.DoubleRow,
                    )
                    nc.tensor.matmul(
                        acc_rs[:], ones_dr[:], e_p[:],
                        start=(p == 0), stop=(p == NP - 1),
                        perf_mode=mybir.MatmulPerfMode.DoubleRow,
                    )

                    if p == NP - 1:
                        # Last i-block: halve the serial recip->mul->add->DMA
                        # tail by processing two 256-column halves.
                        nh = 2 if ib == N_IB - 1 else 1
                        hw_ = IB // nh
                        out_r = out_d.rearrange("(o p) f -> p o f", p=128)
                        for hh in range(nh):
                            sl = slice(hh * hw_, (hh + 1) * hw_)
                            r_t = wp.tile([128, IB], F32, name="r_t", tag="R", bufs=2)
                            nc.vector.reciprocal_approx_fast(r_t[:, sl], acc_rs[:, sl])
                            o_ts = []
                            for cc in range(2):
                                o_t = wp.tile([128, IB], F32, name="o_t", tag="osb", bufs=3)
                                o_ts.append(o_t)
                                acc_cc = acc0 if cc == 0 else acc1c
                                nc.vector.tensor_mul(o_t[:, sl], acc_cc[:, sl], r_t[:, sl])
                            for cc in range(2):
                                # (o + bpt_eff) + x_residual; bpt per-partition
                                o_t = o_ts[cc]
                                nc.vector.scalar_tensor_tensor(
                                    o_t[:, sl], o_t[:, sl], bpt[:, cc:cc + 1],
                                    xq[:, cc, ib * IB + hh * hw_:ib * IB + (hh + 1) * hw_],
                                    op0=mybir.AluOpType.add, op1=mybir.AluOpType.add,
                                )
                                nc.sync.dma_start(
                                    out_r[:, cc, ib * IB + hh * hw_:ib * IB + (hh + 1) * hw_],
                                    o_t[:, sl],
                                )

    nc.compile()
    _NC_CACHE[krep] = nc
    return nc


def _prep_maps(x, Wq, bq, Wk, bk, Wv, bv, Wpt, bpt, gamma):
    bf16 = ml_dtypes.bfloat16
    f32 = np.float32
    g = float(np.asarray(gamma).reshape(-1)[0])
    # wq/wk replicated at column offsets 0 and 32 (S^T 2x row-packing);
    # col 16/48 zero (the shift dim, filled on-device).
    wq2 = np.zeros((C, 49), f32)
    wq2[:, 0:DQ] = Wq.T
    wq2[:, 32:32 + DQ] = Wq.T
    wk2 = np.zeros((C, 49), f32)
    wk2[:, 0:DQ] = Wk.T
    wk2[:, 32:32 + DQ] = Wk.T
    bq_col = np.zeros((49, 1), f32)
    bq_col[0:DQ, 0] = bq
    bq_col[32:32 + DQ, 0] = bq
    kb_col = np.zeros((49, 1), f32)
    kb_col[16, 0] = 1.0
    kb_col[48, 0] = 1.0
    # wcat_r = (g * Wpt[:, r-block] @ Wv).T, layout [c', c]
    wpt_g = (g * Wpt).astype(f32)
    wcat1 = (wpt_g[:, :C] @ Wv).T.astype(f32)
    wcat2 = (wpt_g[:, C:] @ Wv).T.astype(f32)
    bpt_eff = (g * (bpt + Wpt @ np.concatenate([bv, bv]))).astype(f32)
    bpt_col = np.ascontiguousarray(bpt_eff.reshape(2, 128).T)

    xf = np.asarray(x, f32).reshape(B, 2, C, HW)
    xb = xf.astype(bf16)
    def interleave(w):  # [C, F] -> [128, 2, F] partition-major (1 desc/row)
        return np.ascontiguousarray(
            w.astype(bf16).reshape(2, 128, -1).transpose(1, 0, 2))

    wpack1 = interleave(np.concatenate([wq2, wk2, wcat1, wcat2], axis=1))
    wpack2 = interleave(np.concatenate([wq2, wk2, wcat2, wcat1], axis=1))

    # per-query S rowmax (fp32, shared by the two query-half cores of (b,s));
    # any value near the true rowmax works - it only conditions fp8 range.
    mrow = np.empty((B, 2, HW), f32)
    for b in range(B):
        for s in range(2):
            q = Wq @ xf[b, s] + bq.reshape(-1, 1)
            k = Wk @ xf[b, s]
            mrow[b, s] = (q.T @ k).max(axis=1)

    in_maps = []
    for core in range(8):
        b, s, h = core >> 2, (core >> 1) & 1, core & 1
        # joint (x1,x2)/(w1,w2) swap for s=1: M invariant, x1b = own stream.
        # Columns rotated so the own query half leads; attention is
        # j-permutation invariant (K, M^T, rowsum all share the order).
        rot = lambda a: np.ascontiguousarray(np.roll(a, -h * HALF, axis=1))
        in_maps.append(
            dict(
                x1b=rot(xb[b, s]),
                x2b=rot(xb[b, 1 - s]),
                mrow=np.ascontiguousarray(
                    (-mrow[b, s, h * HALF:(h + 1) * HALF]).astype(bf16).reshape(1, HALF)),
                wpack=(wpack1 if s == 0 else wpack2),
                bq_col=bq_col, kb_col=kb_col, bpt_col=bpt_col,
            )
        )
    return in_maps


def kernel(x, Wq, bq, Wk, bk, Wv, bv, Wpt, bpt, gamma, _trace=False):
    from concourse.bass_utils import run_bass_kernel_spmd

    nc = build_bass()
    in_maps = _prep_maps(x, Wq, bq, Wk, bk, Wv, bv, Wpt, bpt, gamma)
    res = run_bass_kernel_spmd(nc, in_maps, list(range(8)), trace=_trace)

    out = np.empty((B, 2, C, HW), np.float32)
    for core in range(8):
        b, s, h = core >> 2, (core >> 1) & 1, core & 1
        out[b, s, :, h * HALF:(h + 1) * HALF] = res.results[core]["out"]
    full = out.reshape(B, 2 * C, 64, 64)
    if _trace:
        return full, res
    return full


# revision 19
# speedup vs baseline: 2.2894x; 1.0092x over previous
"""Trainium2 Bass kernel for nn_Cross_SelfAttention (B=2, C=256, H=W=64, DQ=16).

Sharding: 8 cores = (batch b) x (attn stream s) x (query half h).

Algebraic restructure: the output 1x1 conv is linear, so
  Wpt @ [attn@v1; attn@v2] = (Wpt1@V1 + Wpt2@V2) @ attn^T = M @ attn^T.
M = [C, HW] folds Wv, Wpt, gamma and the 512->256 projection into ONE
256-channel attention apply. M^T is quantized to fp8-e4m3 and the
attention apply runs as DoubleRow fp8 matmuls: one matmul contracts 256
j-positions (a pair of j-chunks), halving PE streaming time again.

fp8 range control: softmax is invariant to a per-query shift of S, so the
host computes m_i ~= max_j S[i, j] and the kernel folds it in as a 17th
contraction dim of the S^T matmul (q16 = -m_i via DMA, k16 = +1 via the
copy bias).  E' = exp(S - m_i + 5) then spans [~0, e^5] - comfortably
inside e4m3.  The shift cancels exactly in acc/rowsum.

Per core:
    M^T[j, c] = x1^T @ wcat1 + x2^T @ wcat2   (bf16 MMs, fp8 output)
    k = Wk @ x_own, q = Wq @ x_own[:, half] + bq   (bf16)
    S'[j, i] = k[:, j].q[:, i] - m_i   (pairs of j-chunks, 2x row packing)
    E' = exp(S' + 5)  (one ACT per 2-bank pair, fp8-e4m3 out)
    acc[c, i] += M^T_pair^T @ E'   (DoubleRow, K=256)
    rowsum    += ones^T @ E'       (DoubleRow, broadcast over partitions)
    out = acc * recip_fast(rowsum) + bpt_eff + x_residual(bf16)
bv is folded into bpt_eff on the host (normalization makes the missing
V-bias contribution exactly Wpt @ [bv; bv]); gamma into wcat/bpt. For
s=1 cores the host swaps (x1b,x2b) AND (w1,w2) jointly - M is invariant
and x1b is always the core's own attention stream.

Each core writes a disjoint [256, 2048] slice of the output; no
collectives needed.
"""

import os

import numpy as np
import ml_dtypes

import concourse.bass as bass
import concourse.bacc as bacc
import concourse.mybir as mybir
from concourse.tile import TileContext
from concourse.bass import ts

BF16 = mybir.dt.bfloat16
F32 = mybir.dt.float32
FP8 = mybir.dt.float8e4

B, C, HW, DQ = 2, 256, 4096, 16
HALF = HW // 2          # query positions per core
IB = 512                # i-block size (one PSUM bank at fp32)
N_IB = HALF // IB       # 4 i-blocks
N_JC = HW // 128        # 32 j-chunks
NP = N_JC // 2          # 16 j-chunk pairs per i-block
EBIAS = 5.0             # E' = exp(S - m_i + EBIAS), max ~e^5 << e4m3 max 448

_NC_CACHE = {}

KREP = int(os.environ.get("KREP", "1"))


def build_bass(krep=None):
    krep = KREP if krep is None else krep
    if krep in _NC_CACHE:
        return _NC_CACHE[krep]

    nc = bacc.Bacc("TRN2", target_bir_lowering=False, debug=False, num_devices=8)

    # Per-core inputs.
    # x1b/x2b are column-rotated per core so the own query half sits at
    # columns 0:HALF (attention is j-permutation invariant when K, M and
    # rowsum share the order) - Q proj and the residual read x1 directly.
    x1_d = nc.dram_tensor("x1b", [C, HW], BF16, kind="ExternalInput")
    x2_d = nc.dram_tensor("x2b", [C, HW], BF16, kind="ExternalInput")
    m_d = nc.dram_tensor("mrow", [1, HALF], BF16, kind="ExternalInput")
    # packed weights: [wq2(49) | wk2(49) | wcat1(256) | wcat2(256)] = 610 cols,
    # pre-interleaved on host to [128 partitions, 2*610] for 1-descriptor rows
    wp_d = nc.dram_tensor("wpack", [128, 2, 610], BF16, kind="ExternalInput")
    bq_d = nc.dram_tensor("bq_col", [49, 1], F32, kind="ExternalInput")
    kb_d = nc.dram_tensor("kb_col", [49, 1], F32, kind="ExternalInput")
    bpt_d = nc.dram_tensor("bpt_col", [128, 2], F32, kind="ExternalInput")
    out_d = nc.dram_tensor("out", [C, HALF], F32, kind="ExternalOutput")

    with TileContext(nc) as tc:
        with (
            tc.tile_pool(name="persist", bufs=1) as pp,
            tc.tile_pool(name="work", bufs=1) as wp,
            tc.tile_pool(name="psum", bufs=1, space="PSUM") as psp,
        ):
            # ---- persistent SBUF tensors ----
            x1 = pp.tile([128, 2, HW], BF16, name="x1_sb")
            x2 = pp.tile([128, 2, HW], BF16, name="x2_sb")
            xq = x1[:, :, 0:HALF]  # own query half (rotated to the front)
            # M^T in fp8, DoubleRow layout: (j_lane, pair, ko=chunk parity, c)
            mT8 = pp.tile([128, NP, 2, C], FP8, name="mT8_sb")
            wpk = pp.tile([128, 2, 610], BF16, name="wpk_sb")
            bq = pp.tile([49, 1], F32, name="bq_sb")
            kb = pp.tile([49, 1], F32, name="kb_sb")
            bpt = pp.tile([128, 2], F32, name="bpt_sb")
            ones_dr = pp.tile([128, 2, 128], FP8, name="ones_dr")
            ebias = pp.tile([128, 1], F32, name="ebias_sb")
            qsb = pp.tile([49, HALF], BF16, name="qsb")
            ksb = pp.tile([49, HW], BF16, name="ksb")

            nc.vector.memset(ones_dr[:], 1.0)
            nc.vector.memset(ebias[:], EBIAS)

            wq = wpk[:, :, 0:49]
            wk = wpk[:, :, 49:98]
            w1s = wpk[:, :, 98:98 + C]
            w2s = wpk[:, :, 98 + C:98 + 2 * C]

            r128 = lambda ap: ap.rearrange("(o p) f -> p o f", p=128)
            nc.sync.dma_start(bq[:], bq_d[:])
            nc.sync.dma_start(kb[:], kb_d[:])
            nc.sync.dma_start(bpt[:], bpt_d[:])
            for _rep in range(krep):
                # DMA order == consumption order: x1c0 (Q + K proj 0-3),
                # weights, then x2 in 1024-col chunks (M^T j-progressive)
                # interleaved with x1c1 (K 4-7).
                nc.sync.dma_start(x1[:, :, ts(0, 2048)], r128(x1_d)[:, :, ts(0, 2048)])
                nc.sync.dma_start(wpk[:], wp_d[:])
                # -m_i into the 17th q row of both packing replicas
                nc.sync.dma_start(qsb[16:17, :], m_d[:])
                nc.sync.dma_start(qsb[48:49, :], m_d[:])
                nc.sync.dma_start(x2[:, :, ts(0, 1024)], r128(x2_d)[:, :, ts(0, 1024)])
                nc.sync.dma_start(x1[:, :, ts(1, 2048)], r128(x1_d)[:, :, ts(1, 2048)])
                for q4 in range(1, 4):
                    nc.sync.dma_start(
                        x2[:, :, ts(q4, 1024)], r128(x2_d)[:, :, ts(q4, 1024)]
                    )

                def k_proj(p4):
                    # the copy's bias writes k16 = +1 into rows 16/48
                    k_ps = psp.tile([128, 2, IB], F32, name="k_ps", tag="s", bufs=2)
                    for hf in range(2):
                        p8 = 2 * p4 + hf
                        nc.tensor.matmul(
                            k_ps[:49, hf], wk[:, 0], x1[:, 0, ts(p8, IB)],
                            start=True, stop=False,
                        )
                        nc.tensor.matmul(
                            k_ps[:49, hf], wk[:, 1], x1[:, 1, ts(p8, IB)],
                            start=False, stop=True,
                        )
                        nc.vector.tensor_scalar_add(
                            ksb[:, ts(p8, IB)], k_ps[:49, hf], kb[:]
                        )

                def m_chunk(jc):
                    m_ps = psp.tile([128, IB], F32, name="m_ps", tag="acc", bufs=4)
                    for cp in range(4):
                        xs_ = x1 if cp < 2 else x2
                        ws_ = w1s if cp < 2 else w2s
                        o = cp % 2
                        nc.tensor.matmul(
                            m_ps[:, 0:C], xs_[:, o, ts(jc, 128)], ws_[:, o],
                            start=(cp == 0), stop=(cp == 3),
                        )
                    nc.vector.tensor_copy(mT8[:, jc // 2, jc % 2, :], m_ps[:, 0:C])

                def q_proj(p2):
                    # bias on DVE; rows 16/48 hold -m_i (DMA above), so only
                    # 0:16 / 32:48 are written.
                    q_ps = psp.tile([128, 2, IB], F32, name="q_ps", tag="s", bufs=2)
                    for hf in range(2):
                        p4 = 2 * p2 + hf
                        nc.tensor.matmul(
                            q_ps[:49, hf], wq[:, 0], xq[:, 0, ts(p4, IB)],
                            start=True, stop=False,
                        )
                        nc.tensor.matmul(
                            q_ps[:49, hf], wq[:, 1], xq[:, 1, ts(p4, IB)],
                            start=False, stop=True,
                        )
                        nc.vector.tensor_scalar_add(
                            qsb[0:16, ts(p4, IB)], q_ps[0:16, hf], bq[0:16]
                        )
                        nc.vector.tensor_scalar_add(
                            qsb[32:48, ts(p4, IB)], q_ps[32:48, hf], bq[32:48]
                        )

                # PE order follows DMA arrival order
                for p2 in range(2):
                    q_proj(p2)
                for p4 in range(2):
                    k_proj(p4)
                for jc in range(8):
                    m_chunk(jc)
                for p4 in range(2, 4):
                    k_proj(p4)
                for jc in range(8, N_JC):
                    m_chunk(jc)

                # ---- main attention loop: software-pipelined pairs ----
                def issue_st(g):
                    ib, p = divmod(g, NP)
                    s_p = psp.tile([128, 2, IB], F32, name="s_p", tag="s", bufs=2)
                    nc.tensor.matmul(
                        s_p[:, 0], ksb[0:17, ts(2 * p, 128)],
                        qsb[0:17, ts(ib, IB)],
                        start=True, stop=True, tile_position=(0, 0),
                    )
                    nc.tensor.matmul(
                        s_p[:, 1], ksb[32:49, ts(2 * p + 1, 128)],
                        qsb[32:49, ts(ib, IB)],
                        start=True, stop=True, tile_position=(32, 0),
                    )
                    return s_p

                s_cur = issue_st(0)
                for g in range(N_IB * NP):
                    ib, p = divmod(g, NP)
                    if p == 0:
                        # acc_rs first: its slot frees earliest (at recip)
                        acc_rs = psp.tile([128, IB], F32, name="acc_rs", tag="acc", bufs=4)
                        acc0 = psp.tile([128, IB], F32, name="acc0", tag="acc", bufs=4)
                        acc1c = psp.tile([128, IB], F32, name="acc1c", tag="acc", bufs=4)
                    e_p = wp.tile([128, 2, IB], FP8, name="e_p", tag="E", bufs=3)
                    nc.scalar.activation(
                        e_p[:], s_cur[:], mybir.ActivationFunctionType.Exp,
                        bias=ebias[:],
                    )
                    if g + 1 < N_IB * NP:
                        s_cur = issue_st(g + 1)
                    nc.tensor.matmul(
                        acc0[:], mT8[:, p, :, 0:128], e_p[:],
                        start=(p == 0), stop=(p == NP - 1),
                        perf_mode=mybir.MatmulPerfMode.DoubleRow,
                    )
                    nc.tensor.matmul(
                        acc1c[:], mT8[:, p, :, 128:256], e_p[:],
                        start=(p == 0), stop=(p == NP - 1),
                        perf_mode=mybir.MatmulPerfMode.DoubleRow,
                    )
                    nc.tensor.matmul(
                        acc_rs[:], ones_dr[:], e_p[:],
                        start=(p == 0), stop=(p == NP - 1),
                        perf_mode=mybir.MatmulPerfMode.DoubleRow,
                    )

                    if p == NP - 1:
                        # Last i-block: halve the serial recip->mul->add->DMA
                        # tail by processing two 256-column halves.
                        nh = 2 if ib == N_IB - 1 else 1
                        hw_ = IB // nh
                        out_r = out_d.rearrange("(o p) f -> p o f", p=128)
                        for hh in range(nh):
                            sl = slice(hh * hw_, (hh + 1) * hw_)
                            r_t = wp.tile([128, IB], F32, name="r_t", tag="R", bufs=2)
                            nc.vector.reciprocal_approx_fast(r_t[:, sl], acc_rs[:, sl])
                            o_ts = []
                            for cc in range(2):
                                o_t = wp.tile([128, IB], F32, name="o_t", tag="osb", bufs=3)
                                o_ts.append(o_t)
                                acc_cc = acc0 if cc == 0 else acc1c
                                nc.vector.tensor_mul(o_t[:, sl], acc_cc[:, sl], r_t[:, sl])
                            for cc in range(2):
                                # (o + bpt_eff) + x_residual; bpt per-partition
                                o_t = o_ts[cc]
                                nc.vector.scalar_tensor_tensor(
                                    o_t[:, sl], o_t[:, sl], bpt[:, cc:cc + 1],
                                    xq[:, cc, ib * IB + hh * hw_:ib * IB + (hh + 1) * hw_],
                                    op0=mybir.AluOpType.add, op1=mybir.AluOpType.add,
                                )
                                nc.sync.dma_start(
                                    out_r[:, cc, ib * IB + hh * hw_:ib * IB + (hh + 1) * hw_],
                                    o_t[:, sl],
                                )

    nc.compile()
    _NC_CACHE[krep] = nc
    return nc


def _prep_maps(x, Wq, bq, Wk, bk, Wv, bv, Wpt, bpt, gamma):
    bf16 = ml_dtypes.bfloat16
    f32 = np.float32
    g = float(np.asarray(gamma).reshape(-1)[0])
    # wq/wk replicated at column offsets 0 and 32 (S^T 2x row-packing);
    # col 16/48 zero (the shift dim, filled on-device).
    wq2 = np.zeros((C, 49), f32)
    wq2[:, 0:DQ] = Wq.T
    wq2[:, 32:32 + DQ] = Wq.T
    wk2 = np.zeros((C, 49), f32)
    wk2[:, 0:DQ] = Wk.T
    wk2[:, 32:32 + DQ] = Wk.T
    bq_col = np.zeros((49, 1), f32)
    bq_col[0:DQ, 0] = bq
    bq_col[32:32 + DQ, 0] = bq
    kb_col = np.zeros((49, 1), f32)
    kb_col[16, 0] = 1.0
    kb_col[48, 0] = 1.0
    # wcat_r = (g * Wpt[:, r-block] @ Wv).T, layout [c', c]
    wpt_g = (g * Wpt).astype(f32)
    wcat1 = (wpt_g[:, :C] @ Wv).T.astype(f32)
    wcat2 = (wpt_g[:, C:] @ Wv).T.astype(f32)
    bpt_eff = (g * (bpt + Wpt @ np.concatenate([bv, bv]))).astype(f32)
    bpt_col = np.ascontiguousarray(bpt_eff.reshape(2, 128).T)

    xf = np.asarray(x, f32).reshape(B, 2, C, HW)
    xb = xf.astype(bf16)
    def interleave(w):  # [C, F] -> [128, 2, F] partition-major (1 desc/row)
        return np.ascontiguousarray(
            w.astype(bf16).reshape(2, 128, -1).transpose(1, 0, 2))

    wpack1 = interleave(np.concatenate([wq2, wk2, wcat1, wcat2], axis=1))
    wpack2 = interleave(np.concatenate([wq2, wk2, wcat2, wcat1], axis=1))

    # per-query S rowmax (fp32, shared by the two query-half cores of (b,s));
    # any value near the true rowmax works - it only conditions fp8 range.
    mrow = np.empty((B, 2, HW), f32)
    for b in range(B):
        for s in range(2):
            q = Wq @ xf[b, s] + bq.reshape(-1, 1)
            k = Wk @ xf[b, s]
            mrow[b, s] = (q.T @ k).max(axis=1)

    in_maps = []
    for core in range(8):
        b, s, h = core >> 2, (core >> 1) & 1, core & 1
        # joint (x1,x2)/(w1,w2) swap for s=1: M invariant, x1b = own stream.
        # Columns rotated so the own query half leads; attention is
        # j-permutation invariant (K, M^T, rowsum all share the order).
        rot = lambda a: np.ascontiguousarray(np.roll(a, -h * HALF, axis=1))
        in_maps.append(
            dict(
                x1b=rot(xb[b, s]),
                x2b=rot(xb[b, 1 - s]),
                mrow=np.ascontiguousarray(
                    (-mrow[b, s, h * HALF:(h + 1) * HALF]).astype(bf16).reshape(1, HALF)),
                wpack=(wpack1 if s == 0 else wpack2),
                bq_col=bq_col, kb_col=kb_col, bpt_col=bpt_col,
            )
        )
    return in_maps


def kernel(x, Wq, bq, Wk, bk, Wv, bv, Wpt, bpt, gamma, _trace=False):
    from concourse.bass_utils import run_bass_kernel_spmd

    nc = build_bass()
    in_maps = _prep_maps(x, Wq, bq, Wk, bk, Wv, bv, Wpt, bpt, gamma)
    res = run_bass_kernel_spmd(nc, in_maps, list(range(8)), trace=_trace)

    out = np.empty((B, 2, C, HW), np.float32)
    for core in range(8):
        b, s, h = core >> 2, (core >> 1) & 1, core & 1
        out[b, s, :, h * HALF:(h + 1) * HALF] = res.results[core]["out"]
    full = out.reshape(B, 2 * C, 64, 64)
    if _trace:
        return full, res
    return full
